# revision 3
# baseline (speedup 1.0000x reference)
"""Trainium2 Bass kernel for the coupling-spline normalizing-flow log-prob.

Strategy (pure data-parallel over 8 cores, 4096 samples each):
- feature-major spline phase: per-bin tensors live as [128, k, Nc] tiles with
  (dim, bin) on partitions and samples on the free axis. Bin-direction
  reductions/cumsums/gathers are PE matmuls with small host-baked 0/1
  matrices; per-bin elementwise work is DVE/GPSIMD muls + one compare.
- unconditional (lower) splines: entire inverse collapses to a gathered
  Mobius transform x=(a*yc+b)/(c*yc+d), ladj=lc-2*ln|c*yc+d| with 5
  host-precomputed coefficient tables over 32 sub-bins (bin x left/right
  of ym). One compare + one gather matmul.
- conditional (upper) splines: hypernet MLP runs feature-major (zero
  transposes); softmax normalization is deferred past the gather
  (unnormalized-compare trick: yc >= cumh  <=>  (yc+3)*Sh >= L2big^T eh);
  softplus/sigmoid are applied post-gather on [16]-sized data.
- formula phase runs sample-major: gathered quantities are PE-transposed
  into [128 samples, 4, cols] batches so every DVE op uses all 128 lanes.
- all transcendentals come from the single ACT table set
  natural_log_exp_and_others (Exp, Ln, Identity, Copy, Relu, Square);
  reciprocals use the DVE iterative-divide instruction.
"""
import numpy as np
from contextlib import ExitStack

import bass_rust as _bass_rust
import concourse.bass as bass
import concourse.bacc as bacc
import concourse.tile as tile
from concourse import mybir
from concourse.alu_op_type import AluOpType as Op
from concourse.bass import ds
from concourse.bass_utils import run_bass_kernel_spmd
from concourse.hw_specs import get_activation_tables
from concourse.masks import make_identity

F32 = mybir.dt.float32
F32R = mybir.dt.float32r
BF16 = mybir.dt.bfloat16
FP8 = mybir.dt.float8e4
AF = mybir.ActivationFunctionType

ACT_SET = 'natural_log_exp_and_others'   # one table set covering all our funcs


class PinnedBacc(bacc.Bacc):
    """Bacc whose act-table placement only ever picks ACT_SET, so exactly
    one LoadActFuncSet is emitted per CFG entry instead of thrashing
    between exp-only and ln-only sets."""

    def insert_act_table_loads(self):
        has_activation = any(
            isinstance(i, mybir.InstActivation)
            for b in self.main_func.blocks
            for i in b.instructions
        )
        if not has_activation:
            return
        tables = [
            (name, (fns if name == ACT_SET else set()))
            for name, fns in get_activation_tables(self.m.arch).items()
        ]
        _bass_rust.insert_act_table_loads(self, tables)

N, D, B = 32768, 32, 16
SPLIT = D // 2
D2 = D - SPLIT
HID = 10 * D
BOUND = 3.0
MBW = 1e-3; MBH = 1e-3; MD = 1e-3; ML = 0.025
LOG2PI = float(np.log(2.0 * np.pi))
CW = 1.0 - MBW * B
CH = 1.0 - MBH * B
PAD_L = float(np.log(np.expm1(1.0 - 2.0 * MD)))

NCORES = 8
NS = N // NCORES          # samples per core
NC = 512                  # samples per chunk
NCH = NS // NC            # chunks per core
NJ = NC // 128            # 128-sample blocks per chunk


# ---------------------------------------------------------------- host tables

def _softmax64(x):
    e = np.exp(x.astype(np.float64) - x.astype(np.float64).max(-1, keepdims=True))
    return e / e.sum(-1, keepdims=True)


def host_mobius_tables(w_raw, h_raw, d_raw, l_raw):
    """thr [512] and telescoped gather matrix G [512, 80] for one
    unconditional spline: rows (dim, subbin j=0..31), cols (v, dim),
    v in {a, b, c, d, lc}."""
    f8 = np.float64
    w = MBW + CW * _softmax64(w_raw)
    h = MBH + CH * _softmax64(h_raw)
    widths = 2 * BOUND * w
    cumw_k = np.concatenate([np.full((SPLIT, 1), -BOUND, f8),
                             -BOUND + 2 * BOUND * np.cumsum(w, -1)], -1)
    cumw_k[:, -1] = BOUND
    heights = 2 * BOUND * h
    cumh_k = np.concatenate([np.full((SPLIT, 1), -BOUND, f8),
                             -BOUND + 2 * BOUND * np.cumsum(h, -1)], -1)
    cumh_k[:, -1] = BOUND
    dv = MD + np.log1p(np.exp(d_raw.astype(f8)))
    pad = np.full((SPLIT, 1), 1.0 - MD, f8)
    dfull = np.concatenate([pad, dv, pad], -1)
    lam = ML + (1 - 2 * ML) / (1 + np.exp(-l_raw.astype(f8)))

    iw = widths; icw = cumw_k[:, :B]; ih = heights; ich = cumh_k[:, :B]
    il = lam; d0 = dfull[:, :B]; d1 = dfull[:, 1:]
    wb = np.sqrt(d0 / d1)
    wc = (il * d0 + (1 - il) * wb * d1) * iw / ih
    ya = ich; yb = ih + ich
    ym = ((1 - il) * ya + il * wb * yb) / ((1 - il) + il * wb)

    a_l = -il * iw + icw * (wc - 1)
    b_l = il * ya * iw + icw * (ya - wc * ym)
    c_l = wc - 1
    dd_l = ya - wc * ym
    lc_l = np.log(wc * il * (ym - ya) * iw)
    nr = wc - il * wb
    a_r = iw * nr + icw * (wc - wb)
    b_r = iw * (il * wb * yb - wc * ym) + icw * (wb * yb - wc * ym)
    c_r = wc - wb
    dd_r = wb * yb - wc * ym
    lc_r = np.log(wb * wc * (1 - il) * (yb - ym) * iw)

    thr = np.zeros((SPLIT, 2 * B), f8)
    vals = np.zeros((5, SPLIT, 2 * B), f8)
    for b in range(B):
        thr[:, 2 * b] = cumh_k[:, b] if b > 0 else -1e30
        thr[:, 2 * b + 1] = ym[:, b]
        for vi, (vl, vr) in enumerate([(a_l, a_r), (b_l, b_r), (c_l, c_r),
                                       (dd_l, dd_r), (lc_l, lc_r)]):
            vals[vi, :, 2 * b] = vl[:, b]
            vals[vi, :, 2 * b + 1] = vr[:, b]
    G = np.zeros((SPLIT * 2 * B, 5 * SPLIT), np.float64)
    for vi in range(5):
        t = vals[vi]
        dvv = np.concatenate([t[:, :1], t[:, 1:] - t[:, :-1]], -1)
        for dd in range(SPLIT):
            G[dd * 2 * B:(dd + 1) * 2 * B, vi * SPLIT + dd] = dvv[dd]
    return thr.reshape(-1).astype(np.float32), G.astype(np.float32)


def host_fold_W3(W3, b3):
    """Fold dlo/dhi pad+shift into W3/b3. New p-row layout:
    w 0:256 | h 256:512 | dlo 512:768 | dhi 768:1024 | l 1024:1280."""
    s0 = D2 * B; s1 = 2 * D2 * B; s2 = s1 + D2 * (B - 1)
    W3 = W3.astype(np.float64); b3 = b3.astype(np.float64)
    W3w, W3h, W3d, W3l = W3[:, :s0], W3[:, s0:s1], W3[:, s1:s2], W3[:, s2:]
    b3w, b3h, b3d, b3l = b3[:s0], b3[s0:s1], b3[s1:s2], b3[s2:]
    P_lo = np.zeros((D2 * (B - 1), D2 * B))
    P_hi = np.zeros((D2 * (B - 1), D2 * B))
    blo = np.zeros(D2 * B)
    bhi = np.zeros(D2 * B)
    for dd in range(D2):
        for b in range(B):
            if b == 0:
                blo[dd * B + b] = PAD_L
            else:
                P_lo[dd * (B - 1) + b - 1, dd * B + b] = 1.0
            if b == B - 1:
                bhi[dd * B + b] = PAD_L
            else:
                P_hi[dd * (B - 1) + b, dd * B + b] = 1.0
    W3n = np.concatenate([W3w, W3h, W3d @ P_lo, W3d @ P_hi, W3l], 1)
    b3n = np.concatenate([b3w, b3h, b3d @ P_lo + blo, b3d @ P_hi + bhi, b3l], 0)
    return W3n.astype(np.float32), b3n.astype(np.float32)


def host_struct_mats():
    """L2big [256,256] (cumsum+idx compare rhs), Dm [256,256] (onehot),
    OB [256,32] (cols 0:16 within-group ones, 16:32 idx = ones for b>=1)."""
    L2 = np.zeros((D2 * B, D2 * B), np.float64)
    Dm = np.zeros((D2 * B, D2 * B), np.float32)
    OB = np.zeros((D2 * B, 32), np.float32)
    for dd in range(D2):
        for b in range(B):
            for k in range(B):
                if b > 0:
                    L2[dd * B + k, dd * B + b] = \
                        (2 * BOUND * CH) * float(k < b) + (2 * BOUND * MBH) * b
            Dm[dd * B + b, dd * B + b] = 1.0
            if b + 1 < B:
                Dm[dd * B + b + 1, dd * B + b] = -1.0
            OB[dd * B + b, dd] = 1.0
            if b > 0:
                OB[dd * B + b, 16 + dd] = 1.0
    return L2.astype(np.float32), Dm, OB


def host_constants(inp):
    """All DRAM constant arrays (identical across cores)."""
    c = {}
    L2, Dm, OB = host_struct_mats()
    c['L2T'] = L2.reshape(2, 128, 256).transpose(1, 0, 2).copy()
    c['DmT'] = Dm.reshape(2, 128, 256).transpose(1, 0, 2).copy()
    c['OB'] = OB.reshape(2, 128, 32).transpose(1, 0, 2).copy()
    REP2 = np.zeros((16, 2, 128), np.float32)
    for r in range(256):
        REP2[r // B, r // 128, r % 128] = 1.0
    c['REP2'] = REP2
    REP4 = np.zeros((16, 4, 128), np.float32)
    for r in range(512):
        REP4[r // (2 * B), r // 128, r % 128] = 1.0
    c['REP4'] = REP4
    scale = 10.0 * inp['ds_stds'].astype(np.float64)
    affc = np.zeros((16, 4), np.float32)
    affc[:, 0] = 1.0 / scale[:16]
    affc[:, 1] = -inp['ds_means'].astype(np.float64)[:16] / scale[:16]
    affc[:, 2] = 1.0 / scale[16:]
    affc[:, 3] = -inp['ds_means'].astype(np.float64)[16:] / scale[16:]
    c['affc'] = affc
    lp_aff = -float(np.sum(np.log(scale)))
    cc = lp_aff - 0.5 * D * LOG2PI
    c['CC'] = np.full((128, 1), cc, np.float32)

    for ci, t in enumerate(['t2', 't1']):
        W1 = inp[t + '_W1'].astype(np.float32)          # [16, 320]
        W2 = inp[t + '_W2'].astype(np.float32)          # [320, 320]
        W3n, b3n = host_fold_W3(inp[t + '_W3'], inp[t + '_b3'])
        W2c = np.zeros((128, 3, 320), np.float32)
        W3c = np.zeros((128, 3, 1280), np.float32)
        for k in range(3):
            kk = min(128, 320 - 128 * k)
            W2c[:kk, k, :] = W2[128 * k:128 * k + kk, :]
            W3c[:kk, k, :] = W3n[128 * k:128 * k + kk, :]
        b1c = np.zeros((128, 3), np.float32)
        b2c = np.zeros((128, 3), np.float32)
        for m in range(3):
            mm = min(128, 320 - 128 * m)
            b1c[:mm, m] = inp[t + '_b1'][128 * m:128 * m + mm]
            b2c[:mm, m] = inp[t + '_b2'][128 * m:128 * m + mm]
        b3wh = np.zeros((128, 4), np.float32)
        b3dl = np.zeros((128, 6), np.float32)
        for m in range(4):
            b3wh[:, m] = b3n[128 * m:128 * (m + 1)]
        for m in range(6):
            b3dl[:, m] = b3n[512 + 128 * m:512 + 128 * (m + 1)]
        thr, G = host_mobius_tables(inp[t + '_w'], inp[t + '_h'],
                                    inp[t + '_d'], inp[t + '_l'])
        pre = f'c{ci}_'
        c[pre + 'W1'] = W1
        c[pre + 'W2'] = W2c
        c[pre + 'W3'] = W3c
        c[pre + 'b1'] = b1c
        c[pre + 'b2'] = b2c
        c[pre + 'b3wh'] = b3wh
        c[pre + 'b3dl'] = b3dl
        c[pre + 'gmob'] = G.reshape(4, 128, 80).transpose(1, 0, 2).copy()
        c[pre + 'thr32'] = thr.reshape(4, 128).T.copy()
    return c


# ------------------------------------------------------------- bass program

# const name -> (shape, kind). kind 'bf16'/'fp8' entries carry quantized
# payload in the f32-word pack (2 resp. 4 values per word) and are upcast
# once on device; 'fp8' entries get an extra 128-word block right after the
# payload holding the per-tensor dequant scale replicated per partition.
CSPECS = {
    'L2T': ([128, 2, 256], 'f32'), 'DmT': ([128, 2, 256], 'f32'),
    'OB': ([128, 2, 32], 'f32'), 'REP2': ([16, 2, 128], 'f32'),
    'REP4': ([16, 4, 128], 'f32'), 'affc': ([16, 4], 'f32'),
    'CC': ([128, 1], 'f32'),
}
for _ci in range(2):
    _p = f'c{_ci}_'
    CSPECS.update({
        _p + 'W1': ([16, 320], 'bf16'), _p + 'W2': ([128, 3, 320], 'fp8'),
        _p + 'W3': ([128, 3, 1280], 'fp8'), _p + 'b1': ([128, 3], 'f32'),
        _p + 'b2': ([128, 3], 'f32'), _p + 'b3wh': ([128, 4], 'f32'),
        _p + 'b3dl': ([128, 6], 'f32'),
        _p + 'gmob': ([128, 4, 80], 'f32'), _p + 'thr32': ([128, 4], 'f32'),
    })


def _words(shp, kind):
    sz = int(np.prod(shp))
    if kind == 'bf16':
        return (sz + 1) // 2
    if kind == 'fp8':
        return (sz + 3) // 4 + 128      # payload + replicated dequant scale
    return sz


def pack_layout():
    """(offsets, total f32 words): every const 128-word aligned, total padded
    to a multiple of 8*128 so the per-core shard is 128-word aligned."""
    off = {}
    cur = 0
    for k, (shp, kind) in CSPECS.items():
        off[k] = cur
        cur += (_words(shp, kind) + 127) // 128 * 128
    cur = (cur + 1023) // 1024 * 1024
    return off, cur


def host_pack_consts(consts):
    import ml_dtypes
    off, tot = pack_layout()
    flat = np.zeros(tot, np.float32)
    for k, (shp, kind) in CSPECS.items():
        a = np.ascontiguousarray(consts[k], dtype=np.float32)
        assert list(a.shape) == shp, (k, a.shape, shp)
        if kind == 'bf16':
            w = a.reshape(-1).astype(ml_dtypes.bfloat16).view(np.uint16)
            w = w.view(np.float32)
            flat[off[k]:off[k] + w.size] = w
        elif kind == 'fp8':
            s = 128.0 / max(float(np.abs(a).max()), 1e-30)
            q = (a.reshape(-1) * s).astype(ml_dtypes.float8_e4m3)
            w = q.view(np.uint8)
            pad = (-w.size) % 4
            w = np.concatenate([w, np.zeros(pad, np.uint8)]).view(np.float32)
            nw = w.size
            flat[off[k]:off[k] + nw] = w
            flat[off[k] + nw:off[k] + nw + 128] = 1.0 / s
        else:
            flat[off[k]:off[k] + a.size] = a.reshape(-1)
    return flat


class K:
    """Holds nc + handles during program construction."""

    def __init__(self, ns=NS, use_loop=True, use_f32r=False, use_gather=True,
                 ncores=NCORES):
        self.ns = ns
        self.nch = ns // NC
        self.use_loop = use_loop
        self.use_f32r = use_f32r
        self.use_gather = use_gather
        self.ncores = ncores
        self.nc_ = PinnedBacc(num_devices=ncores if use_gather else None)

    def build(self):
        nc = self.nc_
        self.xdat = nc.declare_dram_parameter("xdat", [self.ns, D // 2], F32, isOutput=False)
        self.coff, self.ctot = pack_layout()
        if self.use_gather:
            assert self.ctot % self.ncores == 0
            self.cpack = nc.declare_dram_parameter(
                "cpack", [self.ctot // self.ncores], F32, isOutput=False)
        else:
            self.cpack = nc.declare_dram_parameter(
                "cpack", [self.ctot], F32, isOutput=False)
        self.out = nc.declare_dram_parameter("out", [self.ns], F32, isOutput=True)

        with tile.TileContext(nc) as tc, ExitStack() as ctx:
            self.tc = tc
            self.emit(ctx)
        return nc

    # -------------------------------------------------------------- helpers

    def pools(self, ctx):
        tc = self.tc
        if self.use_gather:
            self.dp = ctx.enter_context(tc.tile_pool(name="dram", bufs=1, space="DRAM"))
        self.cp = ctx.enter_context(tc.tile_pool(name="consts", bufs=1))
        self.sp = ctx.enter_context(tc.tile_pool(name="sb", bufs=1))
        self.spU = ctx.enter_context(tc.tile_pool(name="sbU", bufs=8))
        self.sp1 = ctx.enter_context(tc.tile_pool(name="sb1", bufs=2))
        self.sps = ctx.enter_context(tc.tile_pool(name="sbs", bufs=1))
        self.pp = ctx.enter_context(tc.tile_pool(name="ps", bufs=4, space="PSUM"))
        self.pp2 = ctx.enter_context(tc.tile_pool(name="ps2", bufs=1, space="PSUM"))
        self.ppm = ctx.enter_context(tc.tile_pool(name="psm", bufs=2, space="PSUM"))

    def load_consts(self):
        nc = self.nc_
        if self.use_gather:
            shard = self.ctot // self.ncores
            inb = self.dp.tile([shard], F32, tag="inb")
            outb = self.dp.tile([self.ctot], F32, tag="outb")
            nc.gpsimd.dma_start(inb[:], self.cpack[:])
            nc.gpsimd.collective_compute(
                "AllGather", Op.bypass,
                replica_groups=[list(range(self.ncores))],
                ins=[inb.opt()], outs=[outb.opt()])
            src = outb
        else:
            src = self.cpack
        self.ct = {}
        for k, (shp, kind) in CSPECS.items():
            tdt = BF16 if kind in ('bf16', 'fp8') else F32
            t = self.cp.tile(list(shp), tdt, tag=k)
            sz = int(np.prod(shp))
            free = int(np.prod(shp[1:])) if len(shp) > 1 else 1
            base = self.coff[k]
            if kind == 'f32':
                flat = src[base:base + sz]
                if len(shp) == 1:
                    view = flat
                elif len(shp) == 2:
                    view = flat.rearrange("(p a) -> p a", p=shp[0])
                else:
                    view = flat.rearrange("(p a b) -> p a b", p=shp[0], a=shp[1])
                nc.sync.dma_start(out=t, in_=view)
            elif kind == 'bf16':
                view = src[base:base + (sz + 1) // 2].bitcast(BF16)[0:sz]
                view = view.rearrange("(p f) -> p f", p=shp[0])
                stg = self.cp.tile([128, 3840], BF16, tag="bfstg")
                sv = stg[0:shp[0], 0:free]
                nc.sync.dma_start(out=sv, in_=view)
                self.v.tensor_copy(t.rearrange("p ... -> p (...)"), sv)
            else:  # fp8 payload + [128,1] dequant scale
                nw = (sz + 3) // 4
                view = src[base:base + nw].bitcast(FP8)[0:sz]
                view = view.rearrange("(p f) -> p f", p=shp[0])
                stg = self.cp.tile([128, 3840], FP8, tag="fp8stg")
                sv = stg[0:shp[0], 0:free]
                nc.sync.dma_start(out=sv, in_=view)
                sc = self.cp.tile([128, 1], F32, tag=k + "_sc")
                nc.sync.dma_start(
                    out=sc, in_=src[base + nw:base + nw + 128].rearrange(
                        "(p a) -> p a", p=128))
                self.v.tensor_scalar(t.rearrange("p ... -> p (...)"), sv,
                                     sc[:, 0:1], None, Op.mult)
            self.ct[k] = t
        self.ctb = {}
        for k in ('OB', 'REP2', 'REP4', 'DmT', 'L2T'):
            tb_ = self.cp.tile(list(CSPECS[k][0]), BF16, tag=k + "_b")
            self.v.tensor_copy(tb_, self.ct[k])
            self.ctb[k] = tb_
        ident = self.cp.tile([128, 128], F32, tag="ident")
        make_identity(nc, ident)
        self.ident = ident
        mdc = self.cp.tile([128, 1], F32, tag="mdc")
        nc.gpsimd.memset(mdc, MD)
        self.mdc = mdc

    # engine shorthands
    @property
    def v(self):
        return self.nc_.vector

    @property
    def s(self):
        return self.nc_.scalar

    @property
    def g(self):
        return self.nc_.gpsimd

    @property
    def pe(self):
        return self.nc_.tensor

    def mm(self, out, lhsT, rhs, **kw):
        if self.use_f32r:
            lhsT = lhsT.bitcast(F32R)
            rhs = rhs.bitcast(F32R)
        self.pe.matmul(out, lhsT, rhs, **kw)

    def tr(self, out, in_, ident):
        if self.use_f32r:
            out = out.bitcast(F32R)
            in_ = in_.bitcast(F32R)
            ident = ident.bitcast(F32R)
        self.pe.transpose(out, in_, ident)

    def scr(self, tag, shape=None, pool=None):
        pool = pool or self.sps
        return pool.tile(shape or [128, NJ, 16], F32, tag=tag, name=tag)

    def scr2(self, tag, pool=None):
        pool = pool or self.sps
        t = pool.tile([128, NJ * 16], F32, tag=tag, name=tag)
        return t, t.rearrange("p (a b) -> p a b", a=NJ)

    # ------------------------------------------------------ formula helpers

    def clip_mask(self, y_ap):
        """yc, mask from feature-major y [16, NC]."""
        yc = self.sp1.tile([16, NC], BF16, tag="yc")
        self.v.tensor_scalar(yc, y_ap, BOUND, -BOUND, Op.min, Op.max)
        m1 = self.sp1.tile([16, NC], F32, tag="m1")
        self.g.tensor_scalar(m1, y_ap, -BOUND, None, Op.is_ge)
        mask = self.sp1.tile([16, NC], F32, tag="mask")
        self.v.scalar_tensor_tensor(mask, y_ap, BOUND, m1, Op.is_le, Op.mult)
        return yc, mask

    def transpose_into(self, dst_psum, j, src_ap, pcount=128):
        """PE-transpose src [pcount, 128] -> dst_psum[:, j, :pcount]."""
        self.tr(dst_psum[:, j, 0:pcount], src_ap,
                          self.ident[0:pcount, 0:pcount])

    def tback(self, xT):
        """sample-major [128, NJ, 16] -> feature-major [16, NC] SBUF."""
        ps = self.pp.tile([16, NJ, 128], F32, tag="pb")
        for j in range(NJ):
            self.tr(ps[:, j, :], xT[:, j, :], self.ident)
        xf = self.sp1.tile([16, NC], BF16, tag="xf", bufs=3)
        self.v.tensor_copy(xf.rearrange("p (a b) -> p a b", a=NJ), ps)
        return xf

    # --------------------------------------------------------- spline parts

    def uncond(self, ci, y_ap, acc, first):
        """Unconditional (Mobius) spline. y_ap: [16, NC] SBUF feature-major.
        Returns xT sample-major [128, NJ, 16] and feature-major xf."""
        nc = self.nc_
        yc, mask = self.clip_mask(y_ap)
        pre = f'c{ci}_'
        gmob, thr = self.ct[pre + 'gmob'], self.ct[pre + 'thr32']
        rep4 = self.ct['REP4']

        ge = self.sp.tile([128, 4, NC], F32, tag="geu")
        cm = self.pp.tile([128, NC], F32, tag="pb")
        rep4b = self.ctb['REP4']
        for q in range(4):
            rp = self.pp.tile([128, NC], F32, tag="pb")
            self.mm(rp, rep4b[:, q, :], yc, start=True, stop=True)
            self.v.tensor_scalar(ge[:, q, :], rp, thr[:, q:q + 1], None, Op.is_ge)
        for q in range(4):
            self.mm(cm[0:80, :], gmob[:, q, :], ge[:, q, :],
                           start=(q == 0), stop=(q == 3))

        # pack: rows 0:80 = mobius coeffs, 96:112 = yc
        cs = self.sp.tile([128, NC], F32, tag="cs2")
        self.v.tensor_copy(cs[0:80, :], cm[0:80, :])
        self.s.copy(cs[96:112, :], yc)
        tb = self.sp.tile([64, NC], F32, tag="tb2")
        self.g.tensor_copy(tb[0:16, :], mask)
        self.g.tensor_copy(tb[32:48, :], y_ap)

        fmp = self.pp.tile([128, NJ, 128], F32, tag="pb")
        fbp = self.pp.tile([128, NJ, 64], F32, tag="pb")
        for j in range(NJ):
            self.transpose_into(fmp, j, cs[:, 128 * j:128 * (j + 1)])
            self.tr(fbp[:, j, :], tb[:, 128 * j:128 * (j + 1)],
                              self.ident[0:64, 0:64])
        FM = self.sp.tile([128, NJ, 128], F32, tag="fm", bufs=3)
        self.v.tensor_copy(FM, fmp)
        FB = self.sp.tile([128, NJ, 64], F32, tag="fb")
        self.v.tensor_copy(FB, fbp)

        sl = lambda T, i: T[:, :, 16 * i:16 * (i + 1)]
        a, b, c, dd, lc = (sl(FM, i) for i in range(5))
        ycT = FM[:, :, 96:112]
        maskT, yT = FB[:, :, 0:16], FB[:, :, 32:48]

        n = self.scr("f_n")
        self.g.tensor_tensor(n, a, ycT, Op.mult)
        self.g.tensor_tensor(n, n, b, Op.add)
        de = self.scr("f_de")
        self.v.tensor_tensor(de, c, ycT, Op.mult)
        self.v.tensor_tensor(de, de, dd, Op.add)
        r = self.scr("f_r")
        self.v.reciprocal(r, de)
        x = self.scr("f_x")
        self.v.tensor_tensor(x, n, r, Op.mult)
        adn = self.scr("f_adn")
        self.v.scalar_tensor_tensor(adn, de, -1.0, de, Op.mult, Op.max)
        lnd = self.scr("f_lnd")
        self.s.activation(lnd, adn, AF.Ln)
        ladj = self.scr("f_ladj")
        self.v.scalar_tensor_tensor(ladj, lnd, -2.0, lc, Op.mult, Op.add)
        self.g.tensor_tensor(ladj, ladj, maskT, Op.mult)
        xT = self.sp1.tile([128, NJ, 16], F32, tag="xTu", name="xTu")
        self.v.tensor_tensor(xT, x, yT, Op.subtract)
        self.g.tensor_tensor(xT, xT, maskT, Op.mult)
        self.v.tensor_tensor(xT, xT, yT, Op.add)
        self.accum_ladj(ladj, acc, first)
        return xT

    def accum_ladj(self, ladj, acc, first):
        red = self.scr("l_red", [128, NJ])
        self.v.tensor_reduce(red, ladj, mybir.AxisListType.X, Op.add)
        if first:
            self.v.tensor_copy(acc, red)
        else:
            self.v.tensor_tensor(acc, acc, red, Op.add)

    def mlp(self, ci, xf):
        """Hypernet; returns (ew, eh [128,2,NC] SBUF, praw list of 6 PSUM tiles)."""
        nc = self.nc_
        pre = f'c{ci}_'
        W1, W2, W3 = self.ct[pre + 'W1'], self.ct[pre + 'W2'], self.ct[pre + 'W3']
        b1, b2 = self.ct[pre + 'b1'], self.ct[pre + 'b2']
        b3wh = self.ct[pre + 'b3wh']

        h1 = self.sp.tile([128, 3, NC], BF16, tag="h1")
        for m in range(3):
            mm = min(128, 320 - 128 * m)
            ps = self.ppm.tile([128, NC], F32, tag="mlp")
            self.mm(ps[0:mm, :], W1[:, 128 * m:128 * m + mm], xf,
                           start=True, stop=True)
            self.s.activation(h1[0:mm, m, :], ps[0:mm, :], AF.Relu,
                              bias=b1[0:mm, m:m + 1])
        h2 = self.sp.tile([128, 3, NC], BF16, tag="h2")
        for m in range(3):
            mm = min(128, 320 - 128 * m)
            ps = self.ppm.tile([128, NC], F32, tag="mlp")
            for k in range(3):
                kk = min(128, 320 - 128 * k)
                self.mm(ps[0:mm, :], W2[0:kk, k, 128 * m:128 * m + mm],
                               h1[0:kk, k, :], start=(k == 0), stop=(k == 2))
            self.s.activation(h2[0:mm, m, :], ps[0:mm, :], AF.Relu,
                              bias=b2[0:mm, m:m + 1])

        def l3tile(m, tag):
            ps = self.ppm.tile([128, NC], F32, tag=tag)
            for k in range(3):
                kk = min(128, 320 - 128 * k)
                self.mm(ps, W3[0:kk, k, 128 * m:128 * (m + 1)],
                               h2[0:kk, k, :], start=(k == 0), stop=(k == 2))
            return ps

        eh = self.sp.tile([128, 2, NC], BF16, tag="eh")
        for i, m in enumerate((2, 3)):
            ps = l3tile(m, "mlp")
            self.s.activation(eh[:, i, :], ps, AF.Exp, bias=b3wh[:, m:m + 1])
        ew = self.sp.tile([128, 2, NC], BF16, tag="ew")
        for i, m in enumerate((0, 1)):
            ps = l3tile(m, "mlp")
            self.s.activation(ew[:, i, :], ps, AF.Exp, bias=b3wh[:, m:m + 1])
        return ew, eh, l3tile

    def cond(self, ci, y_ap, ew, eh, l3tile, acc, first):
        """Conditional spline. Returns xT sample-major [128, NJ, 16]."""
        nc = self.nc_
        pre = f'c{ci}_'
        b3dl = self.ct[pre + 'b3dl']
        L2T, DmT, OB, REP2 = (self.ctb[k] for k in ('L2T', 'DmT', 'OB', 'REP2'))
        yc, mask = self.clip_mask(y_ap)

        # Sw, Sh
        ss = self.pp.tile([64, NC], F32, tag="pb")
        for k in range(2):
            self.mm(ss[0:16, :], OB[:, k, 0:16], ew[:, k, :],
                           start=(k == 0), stop=(k == 1), tile_position=(0, 0))
        for k in range(2):
            self.mm(ss[32:48, :], OB[:, k, 0:16], eh[:, k, :],
                           start=(k == 0), stop=(k == 1), tile_position=(0, 32))
        ssb = self.sp1.tile([64, NC], F32, tag="ssb")
        self.v.tensor_copy(ssb[0:16, :], ss[0:16, :])
        self.v.tensor_copy(ssb[32:48, :], ss[32:48, :])
        rr = self.sp1.tile([64, NC], F32, tag="rr")
        self.v.reciprocal(rr[0:16, :], ssb[0:16, :])
        self.v.reciprocal(rr[32:48, :], ssb[32:48, :])
        # lhs = (yc + 3) * Sh   (Sh copied to a base-0 tile: tt inputs must
        # share partition ranges per walrus samePartitionsAll)
        shb = self.sp1.tile([16, NC], F32, tag="shb")
        self.s.copy(shb, ssb[32:48, :])
        lhs = self.sp1.tile([16, NC], BF16, tag="lhs")
        self.v.scalar_tensor_tensor(lhs, yc, BOUND, shb, Op.add, Op.mult)
        # replicate lhs to 256 rows
        lhsr = self.sp.tile([128, 2, NC], F32, tag="lhsr")
        for q in range(2):
            rp = self.pp.tile([128, NC], F32, tag="pb")
            self.mm(rp, REP2[:, q, :], lhs, start=True, stop=True)
            self.s.copy(lhsr[:, q, :], rp)
        # rhs2 = L2big^T eh ; ge = lhs_rep >= rhs2
        r2 = self.pp2.tile([128, 2, NC], F32, tag="big2")
        for mh in range(2):
            for k in range(2):
                self.mm(r2[:, mh, :], L2T[:, k, 128 * mh:128 * (mh + 1)],
                               eh[:, k, :], start=(k == 0), stop=(k == 1))
        ge = self.sp.tile([128, 2, NC], BF16, tag="gec")
        for q in range(2):
            self.v.tensor_tensor(ge[:, q, :], lhsr[:, q, :], r2[:, q, :], Op.is_ge)
        # onehot
        ohp = self.pp2.tile([128, 2, NC], F32, tag="big2")
        for mh in range(2):
            for k in range(2):
                self.mm(ohp[:, mh, :], DmT[:, k, 128 * mh:128 * (mh + 1)],
                               ge[:, k, :], start=(k == 0), stop=(k == 1))
        oh = self.sp.tile([128, 2, NC], BF16, tag="oh")
        self.v.tensor_copy(oh, ohp)

        # U muls
        U = {}
        for nm, m0, m1, eng in (("U0", ge, ew, self.g), ("U1", oh, ew, self.v),
                                ("U2", ge, eh, self.g), ("U3", oh, eh, self.v)):
            t = self.spU.tile([128, 2, NC], BF16, tag="U")
            eng.tensor_tensor(t, m0, m1, Op.mult)
            U[nm] = t
        for i, nm in enumerate(("U4", "U5", "U6")):
            t = self.spU.tile([128, 2, NC], BF16, tag="U")
            for half in range(2):
                ps = l3tile(4 + 2 * i + half, "mlp")
                self.v.scalar_tensor_tensor(t[:, half, :], ps,
                                            b3dl[:, 2 * i + half:2 * i + half + 1],
                                            oh[:, half, :], Op.add, Op.mult)
            U[nm] = t

        # contraction into Ce / Co
        ce = self.pp.tile([128, NC], F32, tag="pb")
        co = self.pp.tile([128, NC], F32, tag="pb")
        packs = [(ce, 0, U["U0"]), (ce, 32, U["U2"]), (ce, 64, U["U4"]),
                 (ce, 96, U["U6"]), (co, 0, U["U1"]), (co, 32, U["U3"]),
                 (co, 64, U["U5"])]
        for dst, off, u in packs:
            for k in range(2):
                self.mm(dst[off:off + 16, :], OB[:, k, 0:16], u[:, k, :],
                               start=(k == 0), stop=(k == 1),
                               tile_position=(0, off))
        for k in range(2):
            self.mm(co[96:112, :], OB[:, k, 16:32], ge[:, k, :],
                           start=(k == 0), stop=(k == 1), tile_position=(0, 96))

        # normalize-evict using rw = 1/Sw, rh = 1/Sh computed above
        cse = self.sp.tile([128, NC], F32, tag="cse")
        cso = self.sp.tile([128, NC], F32, tag="cso")
        for dst, src in ((cse, ce), (cso, co)):
            self.v.tensor_tensor(dst[0:16, :], src[0:16, :], rr[0:16, :], Op.mult)
            self.v.tensor_tensor(dst[32:48, :], src[32:48, :], rr[32:48, :], Op.mult)
            self.s.copy(dst[64:80, :], src[64:80, :])
            self.s.copy(dst[96:112, :], src[96:112, :])
        tb3 = self.sp.tile([128, NC], F32, tag="tb3")
        self.s.copy(tb3[0:16, :], yc)
        self.g.tensor_copy(tb3[32:48, :], mask)
        self.g.tensor_copy(tb3[64:80, :], y_ap)

        fep = self.pp.tile([128, NJ, 128], F32, tag="pb")
        fop = self.pp.tile([128, NJ, 128], F32, tag="pb")
        ftp = self.pp.tile([128, NJ, 128], F32, tag="pb")
        for j in range(NJ):
            self.transpose_into(fep, j, cse[:, 128 * j:128 * (j + 1)])
            self.transpose_into(fop, j, cso[:, 128 * j:128 * (j + 1)])
            self.transpose_into(ftp, j, tb3[:, 128 * j:128 * (j + 1)])
        FE = self.sp.tile([128, NJ, 128], F32, tag="fm", bufs=3)
        FO = self.sp.tile([128, NJ, 128], F32, tag="fm", bufs=3)
        FT = self.sp.tile([128, NJ, 128], F32, tag="fm", bufs=3)
        self.v.tensor_copy(FE, fep)
        self.v.tensor_copy(FO, fop)
        self.v.tensor_copy(FT, ftp)

        return self.cond_formula(FE, FO, FT, acc, first)

    def cond_formula(self, FE, FO, FT, acc, first):
        v, s, g = self.v, self.s, self.g
        Ele = FE[:, :, 0:16]; Fle = FE[:, :, 32:48]
        dlo_s = FE[:, :, 64:80]; l_s = FE[:, :, 96:112]
        Eat = FO[:, :, 0:16]; Fat = FO[:, :, 32:48]
        dhi_s = FO[:, :, 64:80]; idx = FO[:, :, 96:112]
        ycT = FT[:, :, 0:16]; maskT = FT[:, :, 32:48]; yT = FT[:, :, 64:80]
        sc = self.scr

        iw = sc("c_iw")
        v.tensor_scalar(iw, Eat, 6 * CW, 6 * MBW, Op.mult, Op.add)
        ih = sc("c_ih")
        v.tensor_scalar(ih, Fat, 6 * CH, 6 * MBH, Op.mult, Op.add)
        elt = sc("c_elt")
        g.tensor_tensor(elt, Ele, Eat, Op.subtract)
        flt = sc("c_flt")
        g.tensor_tensor(flt, Fle, Fat, Op.subtract)
        t0 = sc("c_t0")
        v.tensor_scalar(t0, elt, 6 * CW, -BOUND, Op.mult, Op.add)
        icw = sc("c_icw")
        v.scalar_tensor_tensor(icw, idx, 6 * MBW, t0, Op.mult, Op.add)
        v.tensor_scalar(t0, flt, 6 * CH, -BOUND, Op.mult, Op.add)
        ich = sc("c_ich")
        v.scalar_tensor_tensor(ich, idx, 6 * MBH, t0, Op.mult, Op.add)

        # d0, d1 (softplus), ln d0, ln d1
        e0 = sc("c_e0")
        s.activation(e0, dlo_s, AF.Exp)
        sp0 = sc("c_sp0")
        s.activation(sp0, e0, AF.Ln, bias=1.0)
        ld0 = sc("c_ld0")
        s.activation(ld0, sp0, AF.Ln, bias=self.mdc[:, 0:1])
        d0 = sc("c_d0")
        g.tensor_scalar(d0, sp0, MD, None, Op.add)
        s.activation(e0, dhi_s, AF.Exp)
        sp1t = sc("c_sp1")
        s.activation(sp1t, e0, AF.Ln, bias=1.0)
        ld1 = sc("c_ld1")
        s.activation(ld1, sp1t, AF.Ln, bias=self.mdc[:, 0:1])
        d1 = sc("c_d1")
        g.tensor_scalar(d1, sp1t, MD, None, Op.add)
        wb = sc("c_wb")
        v.tensor_tensor(wb, ld0, ld1, Op.subtract)
        s.activation(wb, wb, AF.Exp, scale=0.5)
        # il
        es = sc("c_es")
        s.activation(es, l_s, AF.Exp, scale=-1.0)
        g.tensor_scalar(es, es, 1.0, None, Op.add)
        il = sc("c_il")
        v.reciprocal(il, es)
        v.tensor_scalar(il, il, 1.0 - 2 * ML, ML, Op.mult, Op.add)

        sm = sc("c_s")
        v.tensor_scalar(sm, il, -1.0, 1.0, Op.mult, Op.add)
        tq = sc("c_t")
        v.tensor_tensor(tq, il, wb, Op.mult)
        rih = sc("c_rih")
        v.reciprocal(rih, ih)
        A = sc("c_A")
        g.tensor_tensor(A, il, d0, Op.mult)
        Bq = sc("c_Bq")
        g.tensor_tensor(Bq, wb, d1, Op.mult)
        g.tensor_tensor(Bq, sm, Bq, Op.mult)
        g.tensor_tensor(A, A, Bq, Op.add)
        wc = sc("c_wc")
        v.tensor_tensor(wc, A, iw, Op.mult)
        v.tensor_tensor(wc, wc, rih, Op.mult)
        yb = sc("c_yb")
        v.tensor_tensor(yb, ih, ich, Op.add)
        mden = sc("c_md")
        v.tensor_tensor(mden, sm, tq, Op.add)
        rm = sc("c_rm")
        v.reciprocal(rm, mden)
        n1 = sc("c_n1")
        g.tensor_tensor(n1, sm, ich, Op.mult)
        n2 = sc("c_n2")
        g.tensor_tensor(n2, tq, yb, Op.mult)
        ym = sc("c_ym")
        v.tensor_tensor(ym, n1, n2, Op.add)
        v.tensor_tensor(ym, ym, rm, Op.mult)
        left = sc("c_left")
        v.tensor_tensor(left, ycT, ym, Op.is_le)
        # num
        numL = sc("c_numL")
        v.tensor_tensor(numL, ich, ycT, Op.subtract)
        v.tensor_tensor(numL, il, numL, Op.mult)
        wcym = sc("c_wcym")
        v.tensor_tensor(wcym, wc, ym, Op.mult)
        q1 = sc("c_q1")
        v.tensor_tensor(q1, wc, tq, Op.subtract)
        v.tensor_tensor(q1, q1, ycT, Op.mult)
        v.tensor_tensor(q1, q1, n2, Op.add)
        v.tensor_tensor(q1, q1, wcym, Op.subtract)
        num = sc("c_num")
        v.tensor_tensor(num, numL, q1, Op.subtract)
        g.tensor_tensor(num, num, left, Op.mult)
        v.tensor_tensor(num, num, q1, Op.add)
        # den
        dl = sc("c_dl")
        v.tensor_scalar(dl, wc, -1.0, None, Op.add)
        v.tensor_tensor(dl, dl, ycT, Op.mult)
        v.tensor_tensor(dl, dl, ich, Op.add)
        v.tensor_tensor(dl, dl, wcym, Op.subtract)
        dr = sc("c_dr")
        v.tensor_tensor(dr, wc, wb, Op.subtract)
        v.tensor_tensor(dr, dr, ycT, Op.mult)
        wbyb = sc("c_wbyb")
        g.tensor_tensor(wbyb, wb, yb, Op.mult)
        v.tensor_tensor(dr, dr, wbyb, Op.add)
        v.tensor_tensor(dr, dr, wcym, Op.subtract)
        den = sc("c_den")
        v.tensor_tensor(den, dl, dr, Op.subtract)
        g.tensor_tensor(den, den, left, Op.mult)
        v.tensor_tensor(den, den, dr, Op.add)
        rden = sc("c_rden")
        v.reciprocal(rden, den)
        xx = sc("c_xx")
        v.tensor_tensor(xx, num, rden, Op.mult)
        v.tensor_tensor(xx, xx, iw, Op.mult)
        v.tensor_tensor(xx, xx, icw, Op.add)
        # dnum
        dnL = sc("c_dnL")
        v.tensor_tensor(dnL, ym, ich, Op.subtract)
        wcil = sc("c_wcil")
        g.tensor_tensor(wcil, wc, il, Op.mult)
        v.tensor_tensor(dnL, wcil, dnL, Op.mult)
        dnR = sc("c_dnR")
        v.tensor_tensor(dnR, yb, ym, Op.subtract)
        wcb = sc("c_wcb")
        g.tensor_tensor(wcb, wc, wb, Op.mult)
        g.tensor_tensor(wcb, wcb, sm, Op.mult)
        v.tensor_tensor(dnR, wcb, dnR, Op.mult)
        dn = sc("c_dn")
        v.tensor_tensor(dn, dnL, dnR, Op.subtract)
        g.tensor_tensor(dn, dn, left, Op.mult)
        v.tensor_tensor(dn, dn, dnR, Op.add)
        v.tensor_tensor(dn, dn, iw, Op.mult)
        adn = sc("c_adn")
        v.scalar_tensor_tensor(adn, den, -1.0, den, Op.mult, Op.max)
        lnn = sc("c_lnn")
        s.activation(lnn, dn, AF.Ln)
        lnd = sc("c_lnd")
        s.activation(lnd, adn, AF.Ln)
        ladj = sc("c_ladj")
        v.scalar_tensor_tensor(ladj, lnd, -2.0, lnn, Op.mult, Op.add)
        v.tensor_tensor(ladj, ladj, maskT, Op.mult)
        xT = self.sp1.tile([128, NJ, 16], F32, tag="xTc", name="xTc")
        v.tensor_tensor(xT, xx, yT, Op.subtract)
        g.tensor_tensor(xT, xT, maskT, Op.mult)
        v.tensor_tensor(xT, xT, yT, Op.add)
        self.accum_ladj(ladj, acc, first)
        return xT

    # --------------------------------------------------------------- emit

    def chunk_body(self, c):
        """Emit one chunk's instructions. `c` is a python int (unrolled) or a
        RuntimeValue chunk index (hardware loop)."""
        nc = self.nc_
        affc = self.ct['affc']
        CCt = self.ct['CC']
        # ---- prep: load + transpose + affine -> z2A/z2B [16, NC]
        zpA = self.pp.tile([16, NJ, 128], F32, tag="pb")
        zpB = self.pp.tile([16, NJ, 128], F32, tag="pb")
        for j in range(NJ):
            xjb = self.sp1.tile([128, D], BF16, tag="xjb")
            nc.sync.dma_start(
                out=xjb, in_=self.xdat[ds(c * NC + 128 * j, 128), :].bitcast(BF16))
            xj = self.sp1.tile([128, D], F32, tag="xj")
            self.g.tensor_copy(xj, xjb)
            self.tr(zpA[:, j, :], xj[:, 0:16], self.ident)
            self.tr(zpB[:, j, :], xj[:, 16:32], self.ident)
        z2A = self.sp.tile([16, NC], F32, tag="z2")
        self.s.activation(z2A.rearrange("p (a b) -> p a b", a=NJ), zpA,
                          AF.Identity, bias=affc[:, 1:2], scale=affc[:, 0:1])
        z2B = self.sp.tile([16, NC], F32, tag="z2b")
        self.s.activation(z2B.rearrange("p (a b) -> p a b", a=NJ), zpB,
                          AF.Identity, bias=affc[:, 3:4], scale=affc[:, 2:3])

        acc = self.sp1.tile([128, NJ], F32, tag="acc")
        # ---- coupling t2 (ci=0)
        x1T_a = self.uncond(0, z2A, acc, first=True)
        x1f_a = self.tback(x1T_a)
        ew, eh, l3t = self.mlp(0, x1f_a)
        x2T_a = self.cond(0, z2B, ew, eh, l3t, acc, first=False)
        x2f_a = self.tback(x2T_a)
        # ---- coupling t1 (ci=1)
        x1T_b = self.uncond(1, x1f_a, acc, first=False)
        x1f_b = self.tback(x1T_b)
        ew, eh, l3t = self.mlp(1, x1f_b)
        x2T_b = self.cond(1, x2f_a, ew, eh, l3t, acc, first=False)

        # ---- finalize
        sq1 = self.scr("sq", [128, NJ, 16])
        self.s.activation(sq1, x1T_b, AF.Square)
        r1 = self.scr("r1", [128, NJ])
        self.v.tensor_reduce(r1, sq1, mybir.AxisListType.X, Op.add)
        self.s.activation(sq1, x2T_b, AF.Square)
        r2 = self.scr("r2", [128, NJ])
        self.v.tensor_reduce(r2, sq1, mybir.AxisListType.X, Op.add)
        logp = self.sp1.tile([128, NJ], F32, tag="logp")
        self.v.tensor_tensor(logp, r1, r2, Op.add)
        self.v.scalar_tensor_tensor(logp, logp, -0.5, acc, Op.mult, Op.add)
        self.v.tensor_scalar(logp, logp, CCt[:, 0:1], None, Op.add)
        ov = self.out[ds(c * NC, NC)].rearrange("(a p) -> p a", p=128)
        nc.sync.dma_start(out=ov, in_=logp)

    def emit(self, ctx):
        self.pools(ctx)
        self.load_consts()
        if self.use_loop and self.nch > 1:
            with self.tc.For_i(0, self.nch) as c:
                self.chunk_body(c)
        else:
            for c in range(self.nch):
                self.chunk_body(c)


_CACHE = {}


def _get_program(ns=NS):
    if ns not in _CACHE:
        k = K(ns)
        nc = k.build()
        nc.finalize()
        _CACHE[ns] = nc
    return _CACHE[ns]


def make_in_maps(inp):
    """(nc-ready in_maps, expected output order) from full inputs."""
    consts = host_constants(inp)
    cflat = host_pack_consts(consts)
    import ml_dtypes
    x = np.ascontiguousarray(inp['data_samples'], dtype=np.float32)
    xb = x.astype(ml_dtypes.bfloat16).view(np.uint16).view(np.float32)
    shard = cflat.size // NCORES
    in_maps = []
    for i in range(NCORES):
        in_maps.append({
            'xdat': xb[i * NS:(i + 1) * NS],
            'cpack': cflat[i * shard:(i + 1) * shard],
        })
    return in_maps


def kernel(**inputs):
    inp = {k: np.asarray(v) for k, v in inputs.items()}
    nc = _get_program(NS)
    in_maps = make_in_maps(inp)
    res = run_bass_kernel_spmd(nc, in_maps, list(range(NCORES)))
    out = np.concatenate([res.results[i]['out'] for i in range(NCORES)])
    return out.astype(np.float32)


if __name__ == '__main__':
    # quick single-core sim check on a small shard
    import jax
    jax.config.update('jax_platforms', 'cpu')
    import reference as ref
    from concourse.bass_interp import CoreSim

    inputs = {k: np.asarray(v) for k, v in ref.setup_inputs().items()}
    consts = host_constants(inputs)
    cflat = host_pack_consts(consts)
    ns = 1024
    k = K(ns, use_gather=False)
    nc = k.build()
    nc.finalize()
    sim = CoreSim(nc, require_finite=False, require_nnan=False)
    import ml_dtypes
    sim.tensor("cpack")[:] = cflat
    xb = inputs['data_samples'][:ns].astype(ml_dtypes.bfloat16).view(np.uint16).view(np.float32)
    sim.tensor("xdat")[:] = xb
    sim.simulate()
    got = np.array(sim.tensor("out"))
    exp = np.asarray(ref.reference(**inputs))[:ns]
    rel = np.linalg.norm(got - exp) / np.linalg.norm(exp)
    print("sim out[:5]", got[:5])
    print("exp    [:5]", exp[:5])
    print("rel l2 err", rel, "max abs", np.abs(got - exp).max())



# revision 4
# speedup vs baseline: 1.2629x; 1.2629x over previous
"""Trainium2 Bass kernel for the coupling-spline normalizing-flow log-prob.

Strategy (pure data-parallel over 8 cores, 4096 samples each):
- feature-major spline phase: per-bin tensors live as [128, k, Nc] tiles with
  (dim, bin) on partitions and samples on the free axis. Bin-direction
  reductions/cumsums/gathers are PE matmuls with small host-baked 0/1
  matrices; per-bin elementwise work is DVE/GPSIMD muls + one compare.
- unconditional (lower) splines: entire inverse collapses to a gathered
  Mobius transform x=(a*yc+b)/(c*yc+d), ladj=lc-2*ln|c*yc+d| with 5
  host-precomputed coefficient tables over 32 sub-bins (bin x left/right
  of ym). One compare + one gather matmul.
- conditional (upper) splines: hypernet MLP runs feature-major (zero
  transposes); softmax normalization is deferred past the gather
  (unnormalized-compare trick: yc >= cumh  <=>  (yc+3)*Sh >= L2big^T eh);
  softplus/sigmoid are applied post-gather on [16]-sized data.
- formula phase runs sample-major: gathered quantities are PE-transposed
  into [128 samples, 4, cols] batches so every DVE op uses all 128 lanes.
- all transcendentals come from the single ACT table set
  natural_log_exp_and_others (Exp, Ln, Identity, Copy, Relu, Square);
  reciprocals use the DVE iterative-divide instruction.
"""
import numpy as np
from contextlib import ExitStack

import bass_rust as _bass_rust
import concourse.bass as bass
import concourse.bacc as bacc
import concourse.tile as tile
from concourse import mybir
from concourse.alu_op_type import AluOpType as Op
from concourse.bass import ds
from concourse.bass_utils import run_bass_kernel_spmd
from concourse.hw_specs import get_activation_tables
from concourse.masks import make_identity

F32 = mybir.dt.float32
F32R = mybir.dt.float32r
BF16 = mybir.dt.bfloat16
FP8 = mybir.dt.float8e4
AF = mybir.ActivationFunctionType

ACT_SET = 'natural_log_exp_and_others'   # one table set covering all our funcs


class PinnedBacc(bacc.Bacc):
    """Bacc whose act-table placement only ever picks ACT_SET, so exactly
    one LoadActFuncSet is emitted per CFG entry instead of thrashing
    between exp-only and ln-only sets."""

    def insert_act_table_loads(self):
        has_activation = any(
            isinstance(i, mybir.InstActivation)
            for b in self.main_func.blocks
            for i in b.instructions
        )
        if not has_activation:
            return
        tables = [
            (name, (fns if name == ACT_SET else set()))
            for name, fns in get_activation_tables(self.m.arch).items()
        ]
        _bass_rust.insert_act_table_loads(self, tables)

N, D, B = 32768, 32, 16
SPLIT = D // 2
D2 = D - SPLIT
HID = 10 * D
BOUND = 3.0
MBW = 1e-3; MBH = 1e-3; MD = 1e-3; ML = 0.025
LOG2PI = float(np.log(2.0 * np.pi))
CW = 1.0 - MBW * B
CH = 1.0 - MBH * B
PAD_L = float(np.log(np.expm1(1.0 - 2.0 * MD)))

NCORES = 8
NS = N // NCORES          # samples per core
NC = 512                  # samples per chunk
NCH = NS // NC            # chunks per core
NJ = NC // 128            # 128-sample blocks per chunk


# ---------------------------------------------------------------- host tables

def _softmax64(x):
    e = np.exp(x.astype(np.float64) - x.astype(np.float64).max(-1, keepdims=True))
    return e / e.sum(-1, keepdims=True)


def host_mobius_tables(w_raw, h_raw, d_raw, l_raw):
    """thr [512] and telescoped gather matrix G [512, 80] for one
    unconditional spline: rows (dim, subbin j=0..31), cols (v, dim),
    v in {a, b, c, d, lc}."""
    f8 = np.float64
    w = MBW + CW * _softmax64(w_raw)
    h = MBH + CH * _softmax64(h_raw)
    widths = 2 * BOUND * w
    cumw_k = np.concatenate([np.full((SPLIT, 1), -BOUND, f8),
                             -BOUND + 2 * BOUND * np.cumsum(w, -1)], -1)
    cumw_k[:, -1] = BOUND
    heights = 2 * BOUND * h
    cumh_k = np.concatenate([np.full((SPLIT, 1), -BOUND, f8),
                             -BOUND + 2 * BOUND * np.cumsum(h, -1)], -1)
    cumh_k[:, -1] = BOUND
    dv = MD + np.log1p(np.exp(d_raw.astype(f8)))
    pad = np.full((SPLIT, 1), 1.0 - MD, f8)
    dfull = np.concatenate([pad, dv, pad], -1)
    lam = ML + (1 - 2 * ML) / (1 + np.exp(-l_raw.astype(f8)))

    iw = widths; icw = cumw_k[:, :B]; ih = heights; ich = cumh_k[:, :B]
    il = lam; d0 = dfull[:, :B]; d1 = dfull[:, 1:]
    wb = np.sqrt(d0 / d1)
    wc = (il * d0 + (1 - il) * wb * d1) * iw / ih
    ya = ich; yb = ih + ich
    ym = ((1 - il) * ya + il * wb * yb) / ((1 - il) + il * wb)

    a_l = -il * iw + icw * (wc - 1)
    b_l = il * ya * iw + icw * (ya - wc * ym)
    c_l = wc - 1
    dd_l = ya - wc * ym
    lc_l = np.log(wc * il * (ym - ya) * iw)
    nr = wc - il * wb
    a_r = iw * nr + icw * (wc - wb)
    b_r = iw * (il * wb * yb - wc * ym) + icw * (wb * yb - wc * ym)
    c_r = wc - wb
    dd_r = wb * yb - wc * ym
    lc_r = np.log(wb * wc * (1 - il) * (yb - ym) * iw)

    thr = np.zeros((SPLIT, 2 * B), f8)
    vals = np.zeros((5, SPLIT, 2 * B), f8)
    for b in range(B):
        thr[:, 2 * b] = cumh_k[:, b] if b > 0 else -1e30
        thr[:, 2 * b + 1] = ym[:, b]
        for vi, (vl, vr) in enumerate([(a_l, a_r), (b_l, b_r), (c_l, c_r),
                                       (dd_l, dd_r), (lc_l, lc_r)]):
            vals[vi, :, 2 * b] = vl[:, b]
            vals[vi, :, 2 * b + 1] = vr[:, b]
    G = np.zeros((SPLIT * 2 * B, 5 * SPLIT), np.float64)
    for vi in range(5):
        t = vals[vi]
        dvv = np.concatenate([t[:, :1], t[:, 1:] - t[:, :-1]], -1)
        for dd in range(SPLIT):
            G[dd * 2 * B:(dd + 1) * 2 * B, vi * SPLIT + dd] = dvv[dd]
    return thr.reshape(-1).astype(np.float32), G.astype(np.float32)


def host_fold_W3(W3, b3):
    """Fold dlo/dhi pad+shift into W3/b3. New p-row layout:
    w 0:256 | h 256:512 | dlo 512:768 | dhi 768:1024 | l 1024:1280."""
    s0 = D2 * B; s1 = 2 * D2 * B; s2 = s1 + D2 * (B - 1)
    W3 = W3.astype(np.float64); b3 = b3.astype(np.float64)
    W3w, W3h, W3d, W3l = W3[:, :s0], W3[:, s0:s1], W3[:, s1:s2], W3[:, s2:]
    b3w, b3h, b3d, b3l = b3[:s0], b3[s0:s1], b3[s1:s2], b3[s2:]
    P_lo = np.zeros((D2 * (B - 1), D2 * B))
    P_hi = np.zeros((D2 * (B - 1), D2 * B))
    blo = np.zeros(D2 * B)
    bhi = np.zeros(D2 * B)
    for dd in range(D2):
        for b in range(B):
            if b == 0:
                blo[dd * B + b] = PAD_L
            else:
                P_lo[dd * (B - 1) + b - 1, dd * B + b] = 1.0
            if b == B - 1:
                bhi[dd * B + b] = PAD_L
            else:
                P_hi[dd * (B - 1) + b, dd * B + b] = 1.0
    W3n = np.concatenate([W3w, W3h, W3d @ P_lo, W3d @ P_hi, W3l], 1)
    b3n = np.concatenate([b3w, b3h, b3d @ P_lo + blo, b3d @ P_hi + bhi, b3l], 0)
    return W3n.astype(np.float32), b3n.astype(np.float32)


def host_struct_mats():
    """L2big [256,256] (cumsum+idx compare rhs), Dm [256,256] (onehot),
    OB [256,32] (cols 0:16 within-group ones, 16:32 idx = ones for b>=1)."""
    L2 = np.zeros((D2 * B, D2 * B), np.float64)
    Dm = np.zeros((D2 * B, D2 * B), np.float32)
    OB = np.zeros((D2 * B, 32), np.float32)
    for dd in range(D2):
        for b in range(B):
            for k in range(B):
                if b > 0:
                    L2[dd * B + k, dd * B + b] = \
                        (2 * BOUND * CH) * float(k < b) + (2 * BOUND * MBH) * b
            Dm[dd * B + b, dd * B + b] = 1.0
            if b + 1 < B:
                Dm[dd * B + b + 1, dd * B + b] = -1.0
            OB[dd * B + b, dd] = 1.0
            if b > 0:
                OB[dd * B + b, 16 + dd] = 1.0
    return L2.astype(np.float32), Dm, OB


def host_constants(inp):
    """All DRAM constant arrays (identical across cores)."""
    c = {}
    L2, Dm, OB = host_struct_mats()
    c['L2T'] = L2.reshape(2, 128, 256).transpose(1, 0, 2).copy()
    c['DmT'] = Dm.reshape(2, 128, 256).transpose(1, 0, 2).copy()
    c['OB'] = OB.reshape(2, 128, 32).transpose(1, 0, 2).copy()
    REP2 = np.zeros((16, 2, 128), np.float32)
    for r in range(256):
        REP2[r // B, r // 128, r % 128] = 1.0
    c['REP2'] = REP2
    REP4 = np.zeros((16, 4, 128), np.float32)
    for r in range(512):
        REP4[r // (2 * B), r // 128, r % 128] = 1.0
    c['REP4'] = REP4
    scale = 10.0 * inp['ds_stds'].astype(np.float64)
    affc = np.zeros((16, 4), np.float32)
    affc[:, 0] = 1.0 / scale[:16]
    affc[:, 1] = -inp['ds_means'].astype(np.float64)[:16] / scale[:16]
    affc[:, 2] = 1.0 / scale[16:]
    affc[:, 3] = -inp['ds_means'].astype(np.float64)[16:] / scale[16:]
    c['affc'] = affc
    lp_aff = -float(np.sum(np.log(scale)))
    cc = lp_aff - 0.5 * D * LOG2PI
    c['CC'] = np.full((128, 1), cc, np.float32)

    for ci, t in enumerate(['t2', 't1']):
        W1 = inp[t + '_W1'].astype(np.float32)          # [16, 320]
        W2 = inp[t + '_W2'].astype(np.float32)          # [320, 320]
        W3n, b3n = host_fold_W3(inp[t + '_W3'], inp[t + '_b3'])
        W2c = np.zeros((128, 3, 320), np.float32)
        W3c = np.zeros((128, 3, 1280), np.float32)
        for k in range(3):
            kk = min(128, 320 - 128 * k)
            W2c[:kk, k, :] = W2[128 * k:128 * k + kk, :]
            W3c[:kk, k, :] = W3n[128 * k:128 * k + kk, :]
        b1c = np.zeros((128, 3), np.float32)
        b2c = np.zeros((128, 3), np.float32)
        for m in range(3):
            mm = min(128, 320 - 128 * m)
            b1c[:mm, m] = inp[t + '_b1'][128 * m:128 * m + mm]
            b2c[:mm, m] = inp[t + '_b2'][128 * m:128 * m + mm]
        b3wh = np.zeros((128, 4), np.float32)
        b3dl = np.zeros((128, 6), np.float32)
        for m in range(4):
            b3wh[:, m] = b3n[128 * m:128 * (m + 1)]
        for m in range(6):
            b3dl[:, m] = b3n[512 + 128 * m:512 + 128 * (m + 1)]
        thr, G = host_mobius_tables(inp[t + '_w'], inp[t + '_h'],
                                    inp[t + '_d'], inp[t + '_l'])
        pre = f'c{ci}_'
        c[pre + 'W1'] = W1
        c[pre + 'W2'] = W2c
        c[pre + 'W3'] = W3c
        c[pre + 'b1'] = b1c
        c[pre + 'b2'] = b2c
        c[pre + 'b3wh'] = b3wh
        c[pre + 'b3dl'] = b3dl
        c[pre + 'gmob'] = G.reshape(4, 128, 80).transpose(1, 0, 2).copy()
        c[pre + 'thr32'] = thr.reshape(4, 128).T.copy()
    return c


# ------------------------------------------------------------- bass program

# const name -> (shape, kind). kind 'bf16'/'fp8' entries carry quantized
# payload in the f32-word pack (2 resp. 4 values per word) and are upcast
# once on device; 'fp8' entries get an extra 128-word block right after the
# payload holding the per-tensor dequant scale replicated per partition.
CSPECS = {
    'L2T': ([128, 2, 256], 'bf16'), 'DmT': ([128, 2, 256], 'bf16'),
    'OB': ([128, 2, 32], 'bf16'), 'REP2': ([16, 2, 128], 'bf16'),
    'REP4': ([16, 4, 128], 'bf16'), 'affc': ([16, 4], 'f32'),
    'CC': ([128, 1], 'f32'),
}
for _ci in range(2):
    _p = f'c{_ci}_'
    CSPECS.update({
        _p + 'W1': ([16, 320], 'bf16'), _p + 'W2': ([128, 3, 320], 'fp8'),
        _p + 'W3': ([128, 3, 1280], 'fp8'), _p + 'b1': ([128, 3], 'f32'),
        _p + 'b2': ([128, 3], 'f32'), _p + 'b3wh': ([128, 4], 'f32'),
        _p + 'b3dl': ([128, 6], 'f32'),
        _p + 'gmob': ([128, 4, 80], 'f32'), _p + 'thr32': ([128, 4], 'f32'),
    })


def _words(shp, kind):
    sz = int(np.prod(shp))
    if kind == 'bf16':
        return (sz + 1) // 2
    if kind == 'fp8':
        return (sz + 3) // 4 + 128      # payload + replicated dequant scale
    return sz


def pack_layout():
    """(offsets, total f32 words): every const 128-word aligned, total padded
    to a multiple of 8*128 so the per-core shard is 128-word aligned."""
    off = {}
    cur = 0
    for k, (shp, kind) in CSPECS.items():
        off[k] = cur
        cur += (_words(shp, kind) + 127) // 128 * 128
    cur = (cur + 1023) // 1024 * 1024
    return off, cur


def host_pack_consts(consts):
    import ml_dtypes
    off, tot = pack_layout()
    flat = np.zeros(tot, np.float32)
    for k, (shp, kind) in CSPECS.items():
        a = np.ascontiguousarray(consts[k], dtype=np.float32)
        assert list(a.shape) == shp, (k, a.shape, shp)
        if kind == 'bf16':
            w = a.reshape(-1).astype(ml_dtypes.bfloat16).view(np.uint16)
            w = w.view(np.float32)
            flat[off[k]:off[k] + w.size] = w
        elif kind == 'fp8':
            s = 128.0 / max(float(np.abs(a).max()), 1e-30)
            q = (a.reshape(-1) * s).astype(ml_dtypes.float8_e4m3)
            w = q.view(np.uint8)
            pad = (-w.size) % 4
            w = np.concatenate([w, np.zeros(pad, np.uint8)]).view(np.float32)
            nw = w.size
            flat[off[k]:off[k] + nw] = w
            flat[off[k] + nw:off[k] + nw + 128] = 1.0 / s
        else:
            flat[off[k]:off[k] + a.size] = a.reshape(-1)
    return flat


class K:
    """Holds nc + handles during program construction."""

    def __init__(self, ns=NS, use_loop=True, use_f32r=False, use_gather=True,
                 ncores=NCORES):
        self.ns = ns
        self.nch = ns // NC
        self.use_loop = use_loop
        self.use_f32r = use_f32r
        self.use_gather = use_gather
        self.ncores = ncores
        self.nc_ = PinnedBacc(num_devices=ncores if use_gather else None)

    def build(self):
        nc = self.nc_
        self.xdat = nc.declare_dram_parameter("xdat", [self.ns, D // 4], F32, isOutput=False)
        self.coff, self.ctot = pack_layout()
        if self.use_gather:
            assert self.ctot % self.ncores == 0
            self.cpack = nc.declare_dram_parameter(
                "cpack", [self.ctot // self.ncores], F32, isOutput=False)
        else:
            self.cpack = nc.declare_dram_parameter(
                "cpack", [self.ctot], F32, isOutput=False)
        self.out = nc.declare_dram_parameter("out", [self.ns], F32, isOutput=True)

        with tile.TileContext(nc) as tc, ExitStack() as ctx:
            self.tc = tc
            self.emit(ctx)
        return nc

    # -------------------------------------------------------------- helpers

    def pools(self, ctx):
        tc = self.tc
        if self.use_gather:
            self.dp = ctx.enter_context(tc.tile_pool(name="dram", bufs=1, space="DRAM"))
        self.cp = ctx.enter_context(tc.tile_pool(name="consts", bufs=1))
        self.sp = ctx.enter_context(tc.tile_pool(name="sb", bufs=1))
        self.spU = ctx.enter_context(tc.tile_pool(name="sbU", bufs=8))
        self.sp1 = ctx.enter_context(tc.tile_pool(name="sb1", bufs=2))
        self.sps = ctx.enter_context(tc.tile_pool(name="sbs", bufs=1))
        self.pp = ctx.enter_context(tc.tile_pool(name="ps", bufs=4, space="PSUM"))
        self.pp2 = ctx.enter_context(tc.tile_pool(name="ps2", bufs=1, space="PSUM"))
        self.ppm = ctx.enter_context(tc.tile_pool(name="psm", bufs=2, space="PSUM"))

    def load_consts(self):
        nc = self.nc_
        if self.use_gather:
            shard = self.ctot // self.ncores
            inb = self.dp.tile([shard], F32, tag="inb")
            outb = self.dp.tile([self.ctot], F32, tag="outb")
            nc.gpsimd.dma_start(inb[:], self.cpack[:])
            nc.gpsimd.collective_compute(
                "AllGather", Op.bypass,
                replica_groups=[list(range(self.ncores))],
                ins=[inb.opt()], outs=[outb.opt()])
            src = outb
        else:
            src = self.cpack
        self.ct = {}
        for k, (shp, kind) in CSPECS.items():
            tdt = BF16 if kind in ('bf16', 'fp8') else F32
            t = self.cp.tile(list(shp), tdt, tag=k)
            sz = int(np.prod(shp))
            free = int(np.prod(shp[1:])) if len(shp) > 1 else 1
            base = self.coff[k]
            if kind == 'f32':
                flat = src[base:base + sz]
                if len(shp) == 1:
                    view = flat
                elif len(shp) == 2:
                    view = flat.rearrange("(p a) -> p a", p=shp[0])
                else:
                    view = flat.rearrange("(p a b) -> p a b", p=shp[0], a=shp[1])
                nc.sync.dma_start(out=t, in_=view)
            elif kind == 'bf16':
                view = src[base:base + (sz + 1) // 2].bitcast(BF16)[0:sz]
                view = view.rearrange("(p f) -> p f", p=shp[0])
                nc.sync.dma_start(out=t.rearrange("p ... -> p (...)"), in_=view)
            else:  # fp8 payload + [128,1] dequant scale
                nw = (sz + 3) // 4
                view = src[base:base + nw].bitcast(FP8)[0:sz]
                view = view.rearrange("(p f) -> p f", p=shp[0])
                stg = self.cp.tile([128, 3840], FP8, tag="fp8stg")
                sv = stg[0:shp[0], 0:free]
                nc.sync.dma_start(out=sv, in_=view)
                sc = self.cp.tile([128, 1], F32, tag=k + "_sc")
                nc.sync.dma_start(
                    out=sc, in_=src[base + nw:base + nw + 128].rearrange(
                        "(p a) -> p a", p=128))
                self.v.tensor_scalar(t.rearrange("p ... -> p (...)"), sv,
                                     sc[:, 0:1], None, Op.mult)
            self.ct[k] = t
        self.ctb = {k: self.ct[k] for k in ('OB', 'REP2', 'REP4', 'DmT', 'L2T')}
        ident = self.cp.tile([128, 128], F32, tag="ident")
        make_identity(nc, ident)
        self.ident = ident
        mdc = self.cp.tile([128, 1], F32, tag="mdc")
        nc.gpsimd.memset(mdc, MD)
        self.mdc = mdc

    # engine shorthands
    @property
    def v(self):
        return self.nc_.vector

    @property
    def s(self):
        return self.nc_.scalar

    @property
    def g(self):
        return self.nc_.gpsimd

    @property
    def pe(self):
        return self.nc_.tensor

    def mm(self, out, lhsT, rhs, **kw):
        if self.use_f32r:
            lhsT = lhsT.bitcast(F32R)
            rhs = rhs.bitcast(F32R)
        self.pe.matmul(out, lhsT, rhs, **kw)

    def tr(self, out, in_, ident):
        if self.use_f32r:
            out = out.bitcast(F32R)
            in_ = in_.bitcast(F32R)
            ident = ident.bitcast(F32R)
        self.pe.transpose(out, in_, ident)

    def scr(self, tag, shape=None, pool=None):
        pool = pool or self.sps
        return pool.tile(shape or [128, NJ, 16], F32, tag=tag, name=tag)

    def scr2(self, tag, pool=None):
        pool = pool or self.sps
        t = pool.tile([128, NJ * 16], F32, tag=tag, name=tag)
        return t, t.rearrange("p (a b) -> p a b", a=NJ)

    # ------------------------------------------------------ formula helpers

    def clip_mask(self, y_ap):
        """yc, mask from feature-major y [16, NC]."""
        yc = self.sp1.tile([16, NC], BF16, tag="yc")
        self.v.tensor_scalar(yc, y_ap, BOUND, -BOUND, Op.min, Op.max)
        m1 = self.sp1.tile([16, NC], F32, tag="m1")
        self.g.tensor_scalar(m1, y_ap, -BOUND, None, Op.is_ge)
        mask = self.sp1.tile([16, NC], F32, tag="mask")
        self.v.scalar_tensor_tensor(mask, y_ap, BOUND, m1, Op.is_le, Op.mult)
        return yc, mask

    def transpose_into(self, dst_psum, j, src_ap, pcount=128):
        """PE-transpose src [pcount, 128] -> dst_psum[:, j, :pcount]."""
        self.tr(dst_psum[:, j, 0:pcount], src_ap,
                          self.ident[0:pcount, 0:pcount])

    def tback(self, xT):
        """sample-major [128, NJ, 16] -> feature-major [16, NC] SBUF."""
        ps = self.pp.tile([16, NJ, 128], F32, tag="pb")
        for j in range(NJ):
            self.tr(ps[:, j, :], xT[:, j, :], self.ident)
        xf = self.sp1.tile([16, NC], BF16, tag="xf", bufs=3)
        self.v.tensor_copy(xf.rearrange("p (a b) -> p a b", a=NJ), ps)
        return xf

    # --------------------------------------------------------- spline parts

    def uncond(self, ci, y_ap, acc, first):
        """Unconditional (Mobius) spline. y_ap: [16, NC] SBUF feature-major.
        Returns xT sample-major [128, NJ, 16] and feature-major xf."""
        nc = self.nc_
        yc, mask = self.clip_mask(y_ap)
        pre = f'c{ci}_'
        gmob, thr = self.ct[pre + 'gmob'], self.ct[pre + 'thr32']
        rep4 = self.ct['REP4']

        ge = self.sp.tile([128, 4, NC], F32, tag="geu")
        cm = self.pp.tile([128, NC], F32, tag="pb")
        rep4b = self.ctb['REP4']
        for q in range(4):
            rp = self.pp.tile([128, NC], F32, tag="pb")
            self.mm(rp, rep4b[:, q, :], yc, start=True, stop=True)
            self.v.tensor_scalar(ge[:, q, :], rp, thr[:, q:q + 1], None, Op.is_ge)
        for q in range(4):
            self.mm(cm[0:80, :], gmob[:, q, :], ge[:, q, :],
                           start=(q == 0), stop=(q == 3))

        # pack: rows 0:80 = mobius coeffs, 96:112 = yc
        cs = self.sp.tile([128, NC], F32, tag="cs2")
        self.v.tensor_copy(cs[0:80, :], cm[0:80, :])
        self.s.copy(cs[96:112, :], yc)
        tb = self.sp.tile([64, NC], F32, tag="tb2")
        self.g.tensor_copy(tb[0:16, :], mask)
        self.g.tensor_copy(tb[32:48, :], y_ap)

        fmp = self.pp.tile([128, NJ, 128], F32, tag="pb")
        fbp = self.pp.tile([128, NJ, 64], F32, tag="pb")
        for j in range(NJ):
            self.transpose_into(fmp, j, cs[:, 128 * j:128 * (j + 1)])
            self.tr(fbp[:, j, :], tb[:, 128 * j:128 * (j + 1)],
                              self.ident[0:64, 0:64])
        FM = self.sp.tile([128, NJ, 128], F32, tag="fm", bufs=3)
        self.v.tensor_copy(FM, fmp)
        FB = self.sp.tile([128, NJ, 64], F32, tag="fb")
        self.v.tensor_copy(FB, fbp)

        sl = lambda T, i: T[:, :, 16 * i:16 * (i + 1)]
        a, b, c, dd, lc = (sl(FM, i) for i in range(5))
        ycT = FM[:, :, 96:112]
        maskT, yT = FB[:, :, 0:16], FB[:, :, 32:48]

        n = self.scr("f_n")
        self.g.tensor_tensor(n, a, ycT, Op.mult)
        self.g.tensor_tensor(n, n, b, Op.add)
        de = self.scr("f_de")
        self.v.tensor_tensor(de, c, ycT, Op.mult)
        self.v.tensor_tensor(de, de, dd, Op.add)
        r = self.scr("f_r")
        self.v.reciprocal(r, de)
        x = self.scr("f_x")
        self.v.tensor_tensor(x, n, r, Op.mult)
        adn = self.scr("f_adn")
        self.v.scalar_tensor_tensor(adn, de, -1.0, de, Op.mult, Op.max)
        lnd = self.scr("f_lnd")
        self.s.activation(lnd, adn, AF.Ln)
        ladj = self.scr("f_ladj")
        self.v.scalar_tensor_tensor(ladj, lnd, -2.0, lc, Op.mult, Op.add)
        self.g.tensor_tensor(ladj, ladj, maskT, Op.mult)
        xT = self.sp1.tile([128, NJ, 16], F32, tag="xTu", name="xTu")
        self.v.tensor_tensor(xT, x, yT, Op.subtract)
        self.g.tensor_tensor(xT, xT, maskT, Op.mult)
        self.v.tensor_tensor(xT, xT, yT, Op.add)
        self.accum_ladj(ladj, acc, first)
        return xT

    def accum_ladj(self, ladj, acc, first):
        red = self.scr("l_red", [128, NJ])
        self.v.tensor_reduce(red, ladj, mybir.AxisListType.X, Op.add)
        if first:
            self.v.tensor_copy(acc, red)
        else:
            self.v.tensor_tensor(acc, acc, red, Op.add)

    def mlp(self, ci, xf):
        """Hypernet; returns (ew, eh [128,2,NC] SBUF, praw list of 6 PSUM tiles)."""
        nc = self.nc_
        pre = f'c{ci}_'
        W1, W2, W3 = self.ct[pre + 'W1'], self.ct[pre + 'W2'], self.ct[pre + 'W3']
        b1, b2 = self.ct[pre + 'b1'], self.ct[pre + 'b2']
        b3wh = self.ct[pre + 'b3wh']

        h1 = self.sp.tile([128, 3, NC], BF16, tag="h1")
        for m in range(3):
            mm = min(128, 320 - 128 * m)
            ps = self.ppm.tile([128, NC], F32, tag="mlp")
            self.mm(ps[0:mm, :], W1[:, 128 * m:128 * m + mm], xf,
                           start=True, stop=True)
            self.s.activation(h1[0:mm, m, :], ps[0:mm, :], AF.Relu,
                              bias=b1[0:mm, m:m + 1])
        h2 = self.sp.tile([128, 3, NC], BF16, tag="h2")
        for m in range(3):
            mm = min(128, 320 - 128 * m)
            ps = self.ppm.tile([128, NC], F32, tag="mlp")
            for k in range(3):
                kk = min(128, 320 - 128 * k)
                self.mm(ps[0:mm, :], W2[0:kk, k, 128 * m:128 * m + mm],
                               h1[0:kk, k, :], start=(k == 0), stop=(k == 2))
            self.s.activation(h2[0:mm, m, :], ps[0:mm, :], AF.Relu,
                              bias=b2[0:mm, m:m + 1])

        def l3tile(m, tag):
            ps = self.ppm.tile([128, NC], F32, tag=tag)
            for k in range(3):
                kk = min(128, 320 - 128 * k)
                self.mm(ps, W3[0:kk, k, 128 * m:128 * (m + 1)],
                               h2[0:kk, k, :], start=(k == 0), stop=(k == 2))
            return ps

        eh = self.sp.tile([128, 2, NC], BF16, tag="eh")
        for i, m in enumerate((2, 3)):
            ps = l3tile(m, "mlp")
            self.s.activation(eh[:, i, :], ps, AF.Exp, bias=b3wh[:, m:m + 1])
        ew = self.sp.tile([128, 2, NC], BF16, tag="ew")
        for i, m in enumerate((0, 1)):
            ps = l3tile(m, "mlp")
            self.s.activation(ew[:, i, :], ps, AF.Exp, bias=b3wh[:, m:m + 1])
        return ew, eh, l3tile

    def cond(self, ci, y_ap, ew, eh, l3tile, acc, first):
        """Conditional spline. Returns xT sample-major [128, NJ, 16]."""
        nc = self.nc_
        pre = f'c{ci}_'
        b3dl = self.ct[pre + 'b3dl']
        L2T, DmT, OB, REP2 = (self.ctb[k] for k in ('L2T', 'DmT', 'OB', 'REP2'))
        yc, mask = self.clip_mask(y_ap)

        # Sw, Sh
        ss = self.pp.tile([64, NC], F32, tag="pb")
        for k in range(2):
            self.mm(ss[0:16, :], OB[:, k, 0:16], ew[:, k, :],
                           start=(k == 0), stop=(k == 1), tile_position=(0, 0))
        for k in range(2):
            self.mm(ss[32:48, :], OB[:, k, 0:16], eh[:, k, :],
                           start=(k == 0), stop=(k == 1), tile_position=(0, 32))
        ssb = self.sp1.tile([64, NC], F32, tag="ssb")
        self.v.tensor_copy(ssb[0:16, :], ss[0:16, :])
        self.v.tensor_copy(ssb[32:48, :], ss[32:48, :])
        rr = self.sp1.tile([64, NC], F32, tag="rr")
        self.v.reciprocal(rr[0:16, :], ssb[0:16, :])
        self.v.reciprocal(rr[32:48, :], ssb[32:48, :])
        # lhs = (yc + 3) * Sh   (Sh copied to a base-0 tile: tt inputs must
        # share partition ranges per walrus samePartitionsAll)
        shb = self.sp1.tile([16, NC], F32, tag="shb")
        self.s.copy(shb, ssb[32:48, :])
        lhs = self.sp1.tile([16, NC], BF16, tag="lhs")
        self.v.scalar_tensor_tensor(lhs, yc, BOUND, shb, Op.add, Op.mult)
        # replicate lhs to 256 rows
        lhsr = self.sp.tile([128, 2, NC], F32, tag="lhsr")
        for q in range(2):
            rp = self.pp.tile([128, NC], F32, tag="pb")
            self.mm(rp, REP2[:, q, :], lhs, start=True, stop=True)
            self.s.copy(lhsr[:, q, :], rp)
        # rhs2 = L2big^T eh ; ge = lhs_rep >= rhs2
        r2 = self.pp2.tile([128, 2, NC], F32, tag="big2")
        for mh in range(2):
            for k in range(2):
                self.mm(r2[:, mh, :], L2T[:, k, 128 * mh:128 * (mh + 1)],
                               eh[:, k, :], start=(k == 0), stop=(k == 1))
        ge = self.sp.tile([128, 2, NC], BF16, tag="gec")
        for q in range(2):
            self.v.tensor_tensor(ge[:, q, :], lhsr[:, q, :], r2[:, q, :], Op.is_ge)
        # onehot
        ohp = self.pp2.tile([128, 2, NC], F32, tag="big2")
        for mh in range(2):
            for k in range(2):
                self.mm(ohp[:, mh, :], DmT[:, k, 128 * mh:128 * (mh + 1)],
                               ge[:, k, :], start=(k == 0), stop=(k == 1))
        oh = self.sp.tile([128, 2, NC], BF16, tag="oh")
        self.v.tensor_copy(oh, ohp)

        # U muls
        U = {}
        for nm, m0, m1, eng in (("U0", ge, ew, self.g), ("U1", oh, ew, self.v),
                                ("U2", ge, eh, self.g), ("U3", oh, eh, self.v)):
            t = self.spU.tile([128, 2, NC], BF16, tag="U")
            eng.tensor_tensor(t, m0, m1, Op.mult)
            U[nm] = t
        for i, nm in enumerate(("U4", "U5", "U6")):
            t = self.spU.tile([128, 2, NC], BF16, tag="U")
            for half in range(2):
                ps = l3tile(4 + 2 * i + half, "mlp")
                self.v.scalar_tensor_tensor(t[:, half, :], ps,
                                            b3dl[:, 2 * i + half:2 * i + half + 1],
                                            oh[:, half, :], Op.add, Op.mult)
            U[nm] = t

        # contraction into Ce / Co
        ce = self.pp.tile([128, NC], F32, tag="pb")
        co = self.pp.tile([128, NC], F32, tag="pb")
        packs = [(ce, 0, U["U0"]), (ce, 32, U["U2"]), (ce, 64, U["U4"]),
                 (ce, 96, U["U6"]), (co, 0, U["U1"]), (co, 32, U["U3"]),
                 (co, 64, U["U5"])]
        for dst, off, u in packs:
            for k in range(2):
                self.mm(dst[off:off + 16, :], OB[:, k, 0:16], u[:, k, :],
                               start=(k == 0), stop=(k == 1),
                               tile_position=(0, off))
        for k in range(2):
            self.mm(co[96:112, :], OB[:, k, 16:32], ge[:, k, :],
                           start=(k == 0), stop=(k == 1), tile_position=(0, 96))

        # normalize-evict using rw = 1/Sw, rh = 1/Sh computed above
        cse = self.sp.tile([128, NC], F32, tag="cse")
        cso = self.sp.tile([128, NC], F32, tag="cso")
        for dst, src in ((cse, ce), (cso, co)):
            self.v.tensor_tensor(dst[0:16, :], src[0:16, :], rr[0:16, :], Op.mult)
            self.v.tensor_tensor(dst[32:48, :], src[32:48, :], rr[32:48, :], Op.mult)
            self.s.copy(dst[64:80, :], src[64:80, :])
            self.s.copy(dst[96:112, :], src[96:112, :])
        tb3 = self.sp.tile([128, NC], F32, tag="tb3")
        self.s.copy(tb3[0:16, :], yc)
        self.g.tensor_copy(tb3[32:48, :], mask)
        self.g.tensor_copy(tb3[64:80, :], y_ap)

        fep = self.pp.tile([128, NJ, 128], F32, tag="pb")
        fop = self.pp.tile([128, NJ, 128], F32, tag="pb")
        ftp = self.pp.tile([128, NJ, 128], F32, tag="pb")
        for j in range(NJ):
            self.transpose_into(fep, j, cse[:, 128 * j:128 * (j + 1)])
            self.transpose_into(fop, j, cso[:, 128 * j:128 * (j + 1)])
            self.transpose_into(ftp, j, tb3[:, 128 * j:128 * (j + 1)])
        FE = self.sp.tile([128, NJ, 128], F32, tag="fm", bufs=3)
        FO = self.sp.tile([128, NJ, 128], F32, tag="fm", bufs=3)
        FT = self.sp.tile([128, NJ, 128], F32, tag="fm", bufs=3)
        self.v.tensor_copy(FE, fep)
        self.v.tensor_copy(FO, fop)
        self.v.tensor_copy(FT, ftp)

        return self.cond_formula(FE, FO, FT, acc, first)

    def cond_formula(self, FE, FO, FT, acc, first):
        v, s, g = self.v, self.s, self.g
        Ele = FE[:, :, 0:16]; Fle = FE[:, :, 32:48]
        dlo_s = FE[:, :, 64:80]; l_s = FE[:, :, 96:112]
        Eat = FO[:, :, 0:16]; Fat = FO[:, :, 32:48]
        dhi_s = FO[:, :, 64:80]; idx = FO[:, :, 96:112]
        ycT = FT[:, :, 0:16]; maskT = FT[:, :, 32:48]; yT = FT[:, :, 64:80]
        sc = self.scr

        iw = sc("c_iw")
        v.tensor_scalar(iw, Eat, 6 * CW, 6 * MBW, Op.mult, Op.add)
        ih = sc("c_ih")
        v.tensor_scalar(ih, Fat, 6 * CH, 6 * MBH, Op.mult, Op.add)
        elt = sc("c_elt")
        g.tensor_tensor(elt, Ele, Eat, Op.subtract)
        flt = sc("c_flt")
        g.tensor_tensor(flt, Fle, Fat, Op.subtract)
        t0 = sc("c_t0")
        v.tensor_scalar(t0, elt, 6 * CW, -BOUND, Op.mult, Op.add)
        icw = sc("c_icw")
        v.scalar_tensor_tensor(icw, idx, 6 * MBW, t0, Op.mult, Op.add)
        v.tensor_scalar(t0, flt, 6 * CH, -BOUND, Op.mult, Op.add)
        ich = sc("c_ich")
        v.scalar_tensor_tensor(ich, idx, 6 * MBH, t0, Op.mult, Op.add)

        # d0, d1 (softplus), ln d0, ln d1
        e0 = sc("c_e0")
        s.activation(e0, dlo_s, AF.Exp)
        sp0 = sc("c_sp0")
        s.activation(sp0, e0, AF.Ln, bias=1.0)
        ld0 = sc("c_ld0")
        s.activation(ld0, sp0, AF.Ln, bias=self.mdc[:, 0:1])
        d0 = sc("c_d0")
        g.tensor_scalar(d0, sp0, MD, None, Op.add)
        s.activation(e0, dhi_s, AF.Exp)
        sp1t = sc("c_sp1")
        s.activation(sp1t, e0, AF.Ln, bias=1.0)
        ld1 = sc("c_ld1")
        s.activation(ld1, sp1t, AF.Ln, bias=self.mdc[:, 0:1])
        d1 = sc("c_d1")
        g.tensor_scalar(d1, sp1t, MD, None, Op.add)
        wb = sc("c_wb")
        v.tensor_tensor(wb, ld0, ld1, Op.subtract)
        s.activation(wb, wb, AF.Exp, scale=0.5)
        # il
        es = sc("c_es")
        s.activation(es, l_s, AF.Exp, scale=-1.0)
        g.tensor_scalar(es, es, 1.0, None, Op.add)
        il = sc("c_il")
        v.reciprocal(il, es)
        v.tensor_scalar(il, il, 1.0 - 2 * ML, ML, Op.mult, Op.add)

        sm = sc("c_s")
        v.tensor_scalar(sm, il, -1.0, 1.0, Op.mult, Op.add)
        tq = sc("c_t")
        v.tensor_tensor(tq, il, wb, Op.mult)
        rih = sc("c_rih")
        v.reciprocal(rih, ih)
        A = sc("c_A")
        g.tensor_tensor(A, il, d0, Op.mult)
        Bq = sc("c_Bq")
        g.tensor_tensor(Bq, wb, d1, Op.mult)
        g.tensor_tensor(Bq, sm, Bq, Op.mult)
        g.tensor_tensor(A, A, Bq, Op.add)
        wc = sc("c_wc")
        v.tensor_tensor(wc, A, iw, Op.mult)
        v.tensor_tensor(wc, wc, rih, Op.mult)
        yb = sc("c_yb")
        v.tensor_tensor(yb, ih, ich, Op.add)
        mden = sc("c_md")
        v.tensor_tensor(mden, sm, tq, Op.add)
        rm = sc("c_rm")
        v.reciprocal(rm, mden)
        n1 = sc("c_n1")
        g.tensor_tensor(n1, sm, ich, Op.mult)
        n2 = sc("c_n2")
        g.tensor_tensor(n2, tq, yb, Op.mult)
        ym = sc("c_ym")
        v.tensor_tensor(ym, n1, n2, Op.add)
        v.tensor_tensor(ym, ym, rm, Op.mult)
        left = sc("c_left")
        v.tensor_tensor(left, ycT, ym, Op.is_le)
        # num
        numL = sc("c_numL")
        v.tensor_tensor(numL, ich, ycT, Op.subtract)
        v.tensor_tensor(numL, il, numL, Op.mult)
        wcym = sc("c_wcym")
        v.tensor_tensor(wcym, wc, ym, Op.mult)
        q1 = sc("c_q1")
        v.tensor_tensor(q1, wc, tq, Op.subtract)
        v.tensor_tensor(q1, q1, ycT, Op.mult)
        v.tensor_tensor(q1, q1, n2, Op.add)
        v.tensor_tensor(q1, q1, wcym, Op.subtract)
        num = sc("c_num")
        v.tensor_tensor(num, numL, q1, Op.subtract)
        g.tensor_tensor(num, num, left, Op.mult)
        v.tensor_tensor(num, num, q1, Op.add)
        # den
        dl = sc("c_dl")
        v.tensor_scalar(dl, wc, -1.0, None, Op.add)
        v.tensor_tensor(dl, dl, ycT, Op.mult)
        v.tensor_tensor(dl, dl, ich, Op.add)
        v.tensor_tensor(dl, dl, wcym, Op.subtract)
        dr = sc("c_dr")
        v.tensor_tensor(dr, wc, wb, Op.subtract)
        v.tensor_tensor(dr, dr, ycT, Op.mult)
        wbyb = sc("c_wbyb")
        g.tensor_tensor(wbyb, wb, yb, Op.mult)
        v.tensor_tensor(dr, dr, wbyb, Op.add)
        v.tensor_tensor(dr, dr, wcym, Op.subtract)
        den = sc("c_den")
        v.tensor_tensor(den, dl, dr, Op.subtract)
        g.tensor_tensor(den, den, left, Op.mult)
        v.tensor_tensor(den, den, dr, Op.add)
        rden = sc("c_rden")
        v.reciprocal(rden, den)
        xx = sc("c_xx")
        v.tensor_tensor(xx, num, rden, Op.mult)
        v.tensor_tensor(xx, xx, iw, Op.mult)
        v.tensor_tensor(xx, xx, icw, Op.add)
        # dnum
        dnL = sc("c_dnL")
        v.tensor_tensor(dnL, ym, ich, Op.subtract)
        wcil = sc("c_wcil")
        g.tensor_tensor(wcil, wc, il, Op.mult)
        v.tensor_tensor(dnL, wcil, dnL, Op.mult)
        dnR = sc("c_dnR")
        v.tensor_tensor(dnR, yb, ym, Op.subtract)
        wcb = sc("c_wcb")
        g.tensor_tensor(wcb, wc, wb, Op.mult)
        g.tensor_tensor(wcb, wcb, sm, Op.mult)
        v.tensor_tensor(dnR, wcb, dnR, Op.mult)
        dn = sc("c_dn")
        v.tensor_tensor(dn, dnL, dnR, Op.subtract)
        g.tensor_tensor(dn, dn, left, Op.mult)
        v.tensor_tensor(dn, dn, dnR, Op.add)
        v.tensor_tensor(dn, dn, iw, Op.mult)
        adn = sc("c_adn")
        v.scalar_tensor_tensor(adn, den, -1.0, den, Op.mult, Op.max)
        lnn = sc("c_lnn")
        s.activation(lnn, dn, AF.Ln)
        lnd = sc("c_lnd")
        s.activation(lnd, adn, AF.Ln)
        ladj = sc("c_ladj")
        v.scalar_tensor_tensor(ladj, lnd, -2.0, lnn, Op.mult, Op.add)
        v.tensor_tensor(ladj, ladj, maskT, Op.mult)
        xT = self.sp1.tile([128, NJ, 16], F32, tag="xTc", name="xTc")
        v.tensor_tensor(xT, xx, yT, Op.subtract)
        g.tensor_tensor(xT, xT, maskT, Op.mult)
        v.tensor_tensor(xT, xT, yT, Op.add)
        self.accum_ladj(ladj, acc, first)
        return xT

    # --------------------------------------------------------------- emit

    def chunk_body(self, c):
        """Emit one chunk's instructions. `c` is a python int (unrolled) or a
        RuntimeValue chunk index (hardware loop)."""
        nc = self.nc_
        affc = self.ct['affc']
        CCt = self.ct['CC']
        # ---- prep: load + transpose + affine -> z2A/z2B [16, NC]
        zpA = self.pp.tile([16, NJ, 128], F32, tag="pb")
        zpB = self.pp.tile([16, NJ, 128], F32, tag="pb")
        for j in range(NJ):
            xjb = self.sp1.tile([128, D], FP8, tag="xjb")
            nc.sync.dma_start(
                out=xjb, in_=self.xdat[ds(c * NC + 128 * j, 128), :].bitcast(FP8))
            xj = self.sp1.tile([128, D], F32, tag="xj")
            self.g.tensor_copy(xj, xjb)
            self.tr(zpA[:, j, :], xj[:, 0:16], self.ident)
            self.tr(zpB[:, j, :], xj[:, 16:32], self.ident)
        z2A = self.sp.tile([16, NC], F32, tag="z2")
        self.s.activation(z2A.rearrange("p (a b) -> p a b", a=NJ), zpA,
                          AF.Identity, bias=affc[:, 1:2], scale=affc[:, 0:1])
        z2B = self.sp.tile([16, NC], F32, tag="z2b")
        self.s.activation(z2B.rearrange("p (a b) -> p a b", a=NJ), zpB,
                          AF.Identity, bias=affc[:, 3:4], scale=affc[:, 2:3])

        acc = self.sp1.tile([128, NJ], F32, tag="acc")
        # ---- coupling t2 (ci=0)
        x1T_a = self.uncond(0, z2A, acc, first=True)
        x1f_a = self.tback(x1T_a)
        ew, eh, l3t = self.mlp(0, x1f_a)
        x2T_a = self.cond(0, z2B, ew, eh, l3t, acc, first=False)
        x2f_a = self.tback(x2T_a)
        # ---- coupling t1 (ci=1)
        x1T_b = self.uncond(1, x1f_a, acc, first=False)
        x1f_b = self.tback(x1T_b)
        ew, eh, l3t = self.mlp(1, x1f_b)
        x2T_b = self.cond(1, x2f_a, ew, eh, l3t, acc, first=False)

        # ---- finalize
        sq1 = self.scr("sq", [128, NJ, 16])
        self.s.activation(sq1, x1T_b, AF.Square)
        r1 = self.scr("r1", [128, NJ])
        self.v.tensor_reduce(r1, sq1, mybir.AxisListType.X, Op.add)
        self.s.activation(sq1, x2T_b, AF.Square)
        r2 = self.scr("r2", [128, NJ])
        self.v.tensor_reduce(r2, sq1, mybir.AxisListType.X, Op.add)
        logp = self.sp1.tile([128, NJ], F32, tag="logp")
        self.v.tensor_tensor(logp, r1, r2, Op.add)
        self.v.scalar_tensor_tensor(logp, logp, -0.5, acc, Op.mult, Op.add)
        self.v.tensor_scalar(logp, logp, CCt[:, 0:1], None, Op.add)
        ov = self.out[ds(c * NC, NC)].rearrange("(a p) -> p a", p=128)
        nc.sync.dma_start(out=ov, in_=logp)

    def emit(self, ctx):
        self.pools(ctx)
        self.load_consts()
        if self.use_loop and self.nch > 1:
            with self.tc.For_i(0, self.nch) as c:
                self.chunk_body(c)
        else:
            for c in range(self.nch):
                self.chunk_body(c)


_CACHE = {}


def _get_program(ns=NS):
    if ns not in _CACHE:
        k = K(ns)
        nc = k.build()
        nc.finalize()
        _CACHE[ns] = nc
    return _CACHE[ns]


def quant_x(x):
    """fp8-e4m3 packed samples (as f32 words) plus the dequant scale."""
    import ml_dtypes
    s = 128.0 / max(float(np.abs(x).max()), 1e-30)
    q = (x.reshape(-1) * s).astype(ml_dtypes.float8_e4m3)
    xb = q.view(np.uint8).view(np.float32).reshape(x.shape[0], -1)
    return xb, s


def make_in_maps(inp):
    """(nc-ready in_maps, expected output order) from full inputs."""
    consts = host_constants(inp)
    x = np.ascontiguousarray(inp['data_samples'], dtype=np.float32)
    xb, s = quant_x(x)
    # on-device xj carries s*x; fold 1/s into the affine scale column
    consts = dict(consts)
    affc = consts['affc'].copy()
    affc[:, 0] /= s
    affc[:, 2] /= s
    consts['affc'] = affc
    cflat = host_pack_consts(consts)
    shard = cflat.size // NCORES
    in_maps = []
    for i in range(NCORES):
        in_maps.append({
            'xdat': xb[i * NS:(i + 1) * NS],
            'cpack': cflat[i * shard:(i + 1) * shard],
        })
    return in_maps


def kernel(**inputs):
    inp = {k: np.asarray(v) for k, v in inputs.items()}
    nc = _get_program(NS)
    in_maps = make_in_maps(inp)
    res = run_bass_kernel_spmd(nc, in_maps, list(range(NCORES)))
    out = np.concatenate([res.results[i]['out'] for i in range(NCORES)])
    return out.astype(np.float32)


if __name__ == '__main__':
    # quick single-core sim check on a small shard
    import jax
    jax.config.update('jax_platforms', 'cpu')
    import reference as ref
    from concourse.bass_interp import CoreSim

    inputs = {k: np.asarray(v) for k, v in ref.setup_inputs().items()}
    consts = host_constants(inputs)
    cflat = host_pack_consts(consts)
    ns = 1024
    k = K(ns, use_gather=False)
    nc = k.build()
    nc.finalize()
    sim = CoreSim(nc, require_finite=False, require_nnan=False)
    x = np.ascontiguousarray(inputs['data_samples'][:ns], dtype=np.float32)
    xb, s = quant_x(x)
    consts = dict(consts)
    affc = consts['affc'].copy()
    affc[:, 0] /= s
    affc[:, 2] /= s
    consts['affc'] = affc
    cflat = host_pack_consts(consts)
    sim.tensor("cpack")[:] = cflat
    sim.tensor("xdat")[:] = xb
    sim.simulate()
    got = np.array(sim.tensor("out"))
    exp = np.asarray(ref.reference(**inputs))[:ns]
    rel = np.linalg.norm(got - exp) / np.linalg.norm(exp)
    print("sim out[:5]", got[:5])
    print("exp    [:5]", exp[:5])
    print("rel l2 err", rel, "max abs", np.abs(got - exp).max())

